# revision 21
# baseline (speedup 1.0000x reference)
"""FAME-GCN Trainium2 kernel, v3.

Computes, for merged adjacency final_A = temp + temp^T, temp = sum_k w_k A_k:
    U1 = final_A @ (feature @ W3) + b3
    U2 = final_A2 @ (feature @ W1) + b1
    out = concat(U1, U2, axis=1)          # [5000, 32]

Distribution: node rows sharded 625/core across 8 NeuronCores; the [16, N]
column-direction partials (temp^T S) are summed across cores on the host,
the row-direction results (temp S)[own rows] concatenate.

Host prep: w_k is folded into a bf16 cast of each adjacency (w_k * A_k),
so the on-device merge is a pure chain of 2x-mode bf16 tensor_adds and the
dir1 stationaries are plain support matrices.

Per core, 5 stripes of 125 rows:
  - the 12 scaled relation stripes arrive as [125, 5000] bf16 transfers
    spread over six DMA paths (sync-HWDGE, scalar-HWDGE, 4 SWDGE queues),
  - dir1 (temp^T S, both groups): per-relation matmuls against a stacked
    [125, 32] stationary (S3-half for A-relations, S1-half for A_t),
    PSUM-accumulated over all 12 relations, one flush per column block,
  - dir2 (temp S): bf16 tensor_add merge per group on DVE, then PE
    transposes (8 column chunks packed per PSUM bank, bulk-copied to SBUF
    by the scalar engine) feeding an S-stationary accumulation chain.
"""

import sys

if "/opt/trn_rl_repo" not in sys.path:
    sys.path.insert(0, "/opt/trn_rl_repo")

import ml_dtypes
import numpy as np

import concourse.bacc as bacc
import concourse.mybir as mybir
from concourse.tile import TileContext
from concourse.bass_utils import run_bass_kernel_spmd

F32 = mybir.dt.float32
BF16 = mybir.dt.bfloat16

N = 5000
NP = 5120  # padded row length for the gather path (row bytes % 256 == 0)
OUT = 16
K_A, K_AT = 3, 9
KTOT = K_A + K_AT
NCORES = 8
RS = N // NCORES  # 625 rows per core
STRIPE = 125
NSTRIPE = RS // STRIPE  # 5
CB = 512
NCB = (N + CB - 1) // CB  # 10
NJC = (N + 127) // 128  # 40
JPACK = 8  # transposed 128-col chunks packed per PSUM bank

_CACHE = {}


def _c_blocks():
    return [(cb * CB, min(CB, N - cb * CB)) for cb in range(NCB)]


def _j_chunks():
    return [(j, min(128, N - j * 128)) for j in range(NJC)]


def build():
    nc = bacc.Bacc(num_swdge_queues=4)

    adjh = nc.declare_dram_parameter("adjh", [K_A, RS, N], BF16, isOutput=False)
    adjg = nc.declare_dram_parameter("adjg", [K_AT, RS, NP], BF16, isOutput=False)
    idxs = nc.declare_dram_parameter("idxs", [128, 8 * NSTRIPE], mybir.dt.int16, isOutput=False)
    sst = nc.declare_dram_parameter("sst", [STRIPE, NSTRIPE * 64], BF16, isOutput=False)
    sfa = nc.declare_dram_parameter("sfa", [128, NJC * OUT], BF16, isOutput=False)
    sfb = nc.declare_dram_parameter("sfb", [128, NJC * OUT], BF16, isOutput=False)
    idt = nc.declare_dram_parameter("idt", [128, 128], BF16, isOutput=False)

    o1 = nc.declare_dram_parameter("o1", [32, N], F32, isOutput=True)
    o2a = nc.declare_dram_parameter("o2a", [OUT, RS], F32, isOutput=True)
    o2b = nc.declare_dram_parameter("o2b", [OUT, RS], F32, isOutput=True)

    # group-a relations (k<3) via sync-HWDGE; group-b via SWDGE gathers
    # spread over the 4 SWDGE queues

    with TileContext(nc) as tc:
        with (
            tc.tile_pool(name="persist", bufs=1) as pp,
            tc.tile_pool(name="raws", bufs=4) as rawsp,
            tc.tile_pool(name="rawg", bufs=10) as rawgp,
            tc.tile_pool(name="mrg", bufs=1) as mrgp,
            tc.tile_pool(name="strip", bufs=2) as stripp,
            tc.tile_pool(name="pdir", bufs=2, space="PSUM") as pdirp,
            tc.tile_pool(name="pt", bufs=2, space="PSUM") as ptp,
            tc.tile_pool(name="pd2", bufs=2, space="PSUM") as pd2p,
        ):
            # ---------------- persistent tiles ----------------
            sst_t = pp.tile([STRIPE, NSTRIPE * 64], BF16, tag="sst")
            nc.sync.dma_start(out=sst_t, in_=sst[:, :])
            sfa_t = pp.tile([128, NJC * OUT], BF16, tag="sfa")
            nc.sync.dma_start(out=sfa_t, in_=sfa[:, :])
            sfb_t = pp.tile([128, NJC * OUT], BF16, tag="sfb")
            nc.sync.dma_start(out=sfb_t, in_=sfb[:, :])
            id_t = pp.tile([128, 128], BF16, tag="idt")
            nc.sync.dma_start(out=id_t, in_=idt[:, :])
            ix = pp.tile([128, 8 * NSTRIPE], mybir.dt.int16, tag="ix")
            nc.sync.dma_start(out=ix, in_=idxs[:, :])

            o1sb = pp.tile([32, N], F32, tag="o1sb")
            acc2a = pp.tile([OUT, RS], F32, tag="acc2a")
            acc2b = pp.tile([OUT, RS], F32, tag="acc2b")

            for st in range(NSTRIPE):
                r0 = st * STRIPE
                # ---- loads: one dma per relation stripe, 5 dma paths ----
                raw = {}
                for k in range(KTOT):
                    if k < K_A:
                        t = rawsp.tile(
                            [STRIPE, N], BF16, tag="traw", name=f"t_{st}_{k}"
                        )
                        nc.sync.dma_start(out=t, in_=adjh[k, r0 : r0 + STRIPE, :])
                        raw[k] = t
                    else:
                        t = rawgp.tile(
                            [128, 1, NP], BF16, tag="trawg", name=f"t_{st}_{k}"
                        )
                        nc.gpsimd.dma_gather(
                            t,
                            adjg[k - K_A, :, :],
                            ix[:, st * 8 : (st + 1) * 8],
                            128,
                            128,
                            NP,
                            elem_step=NP,
                            queue_num=(k - K_A) % 4,
                        )
                        raw[k] = t

                def rawsl(k, c0, cw):
                    t = raw[k]
                    if k < K_A:
                        return t[:, c0 : c0 + cw]
                    return t[:STRIPE, 0, c0 : c0 + cw]

                # ---- dir1: one stacked PSUM chain over all 12 relations ----
                sa = sst_t[:, st * 64 : st * 64 + 32]
                sb = sst_t[:, st * 64 + 32 : st * 64 + 64]
                for cb, (c0, cw) in enumerate(_c_blocks()):
                    pd = pdirp.tile([32, CB], F32, tag="pd", name=f"pd_{st}_{cb}")
                    for k in range(KTOT):
                        nc.tensor.matmul(
                            pd[:, :cw],
                            sa if k < K_A else sb,
                            rawsl(k, c0, cw),
                            start=(k == 0),
                            stop=(k == KTOT - 1),
                        )
                    dst = o1sb[:, c0 : c0 + cw]
                    if st == 0:
                        nc.vector.tensor_copy(out=dst, in_=pd[:, :cw])
                    else:
                        nc.vector.tensor_add(dst, dst, pd[:, :cw])

                # ---- merge per group: mrg = sum_k (w_k A_k) (bf16 adds) ----
                mrga = mrgp.tile([STRIPE, N], BF16, tag="mrga", name=f"mrga_{st}")
                mrgb = mrgp.tile([STRIPE, N], BF16, tag="mrgb", name=f"mrgb_{st}")
                nc.vector.tensor_add(mrga, rawsl(0, 0, N), rawsl(1, 0, N))
                nc.vector.tensor_add(mrga, mrga, rawsl(2, 0, N))
                nc.vector.tensor_add(mrgb, rawsl(3, 0, N), rawsl(4, 0, N))
                for k in range(5, KTOT):
                    nc.vector.tensor_add(mrgb, mrgb, rawsl(k, 0, N))

                # ---- dir2 per group: acc2[:, st] = (mrg @ S)^T ----
                for gname, mrg, sf_t, acc2 in (
                    ("a", mrga, sfa_t, acc2a),
                    ("b", mrgb, sfb_t, acc2b),
                ):
                    pd2 = pd2p.tile(
                        [OUT, 126], F32, tag="pd2", name=f"pd2_{st}_{gname}"
                    )
                    jcs = _j_chunks()
                    for jb in range(0, NJC, JPACK):
                        chunk = jcs[jb : jb + JPACK]
                        pt = ptp.tile(
                            [128, JPACK * 126],
                            BF16,
                            tag="pt",
                            name=f"pt_{st}_{gname}_{jb}",
                        )
                        for jj, (j, cjw) in enumerate(chunk):
                            nc.tensor.transpose(
                                pt[:cjw, jj * 126 : jj * 126 + 126],
                                mrg[:STRIPE, 128 * j : 128 * j + cjw],
                                id_t[:STRIPE, :126],
                            )
                        strip = stripp.tile(
                            [128, JPACK * 126],
                            BF16,
                            tag="tt",
                            name=f"tt_{st}_{gname}_{jb}",
                        )
                        wid = len(chunk) * 126
                        nc.scalar.copy(out=strip[:, :wid], in_=pt[:, :wid])
                        for jj, (j, cjw) in enumerate(chunk):
                            nc.tensor.matmul(
                                pd2[:, :126],
                                sf_t[:cjw, j * OUT : (j + 1) * OUT],
                                strip[:cjw, jj * 126 : jj * 126 + 126],
                                start=(j == 0),
                                stop=(j == NJC - 1),
                            )
                    nc.vector.tensor_copy(
                        out=acc2[:, r0 : r0 + STRIPE], in_=pd2[:, :STRIPE]
                    )

            nc.sync.dma_start(out=o1[:, :], in_=o1sb)
            nc.sync.dma_start(out=o2a[:, :], in_=acc2a)
            nc.sync.dma_start(out=o2b[:, :], in_=acc2b)

    nc.compile()
    return nc


def _make_inputs(feature, A, A_t, w2, wb, W3, W1):
    bf16 = ml_dtypes.bfloat16

    S3 = (feature @ W3).astype(np.float32)  # [N, 16]
    S1 = (feature @ W1).astype(np.float32)

    # sf: S laid out [128, NJC*16]; sf[p, j*16+o] = S[j*128+p, o]
    def make_sf(S):
        sf = np.zeros((128, NJC * OUT), dtype=np.float32)
        for j in range(NJC):
            w = min(128, N - j * 128)
            sf[:w, j * OUT : (j + 1) * OUT] = S[j * 128 : j * 128 + w]
        return sf.astype(bf16)

    sfa = make_sf(S3)
    sfb = make_sf(S1)
    eye = np.eye(128, dtype=bf16)

    # w_k folded into the adjacency cast
    Ascl = (A * w2[:, None, None]).astype(bf16)  # [3, N, N]
    Atscl = (A_t * wb[:, None, None]).astype(bf16)  # [9, N, N]

    idxs = np.full((128, 8 * NSTRIPE), -1, dtype=np.int16)
    for st in range(NSTRIPE):
        for j in range(STRIPE):
            for rep in range(8):
                idxs[j % 16 + 16 * rep, st * 8 + j // 16] = STRIPE * st + j

    in_maps = []
    for p in range(NCORES):
        r0 = p * RS
        adj = np.concatenate(
            [Ascl[:, r0 : r0 + RS, :], Atscl[:, r0 : r0 + RS, :]], axis=0
        )
        adjh = np.ascontiguousarray(adj[:K_A])  # [3, RS, N]
        adjg = np.zeros((K_AT, RS, NP), dtype=bf16)
        adjg[:, :, :N] = adj[K_A:]
        # stacked dir1 stationaries: [125, st*64 + (0:32 sa | 32:64 sb)]
        sst = np.zeros((STRIPE, NSTRIPE * 64), dtype=np.float32)
        for st in range(NSTRIPE):
            rows = slice(r0 + st * STRIPE, r0 + (st + 1) * STRIPE)
            sst[:, st * 64 : st * 64 + OUT] = S3[rows]
            sst[:, st * 64 + 48 : st * 64 + 64] = S1[rows]
        in_maps.append(
            {
                "adjh": adjh,
                "adjg": adjg,
                "idxs": idxs,
                "sst": sst.astype(bf16),
                "sfa": sfa,
                "sfb": sfb,
                "idt": eye,
            }
        )
    return in_maps


def kernel(feature, A, A_t, weight_b2, weight_b, W3, b3, W1, b1, **kw):
    feature = np.asarray(feature, dtype=np.float32)
    A = np.asarray(A, dtype=np.float32)
    A_t = np.asarray(A_t, dtype=np.float32)
    w2 = np.asarray(weight_b2, dtype=np.float32).reshape(K_A)
    wb = np.asarray(weight_b, dtype=np.float32).reshape(K_AT)
    W3 = np.asarray(W3, dtype=np.float32)
    W1 = np.asarray(W1, dtype=np.float32)
    b3 = np.asarray(b3, dtype=np.float32)
    b1 = np.asarray(b1, dtype=np.float32)

    if "nc" not in _CACHE:
        _CACHE["nc"] = build()
    nc = _CACHE["nc"]

    in_maps = _make_inputs(feature, A, A_t, w2, wb, W3, W1)
    _CACHE["in_maps"] = in_maps

    res = run_bass_kernel_spmd(nc, in_maps, core_ids=list(range(NCORES)))

    col_a = np.zeros((OUT, N), dtype=np.float32)
    col_b = np.zeros((OUT, N), dtype=np.float32)
    row_a = np.empty((OUT, N), dtype=np.float32)
    row_b = np.empty((OUT, N), dtype=np.float32)
    for p in range(NCORES):
        r = res.results[p]
        col_a += r["o1"][0:16]
        col_b += r["o1"][16:32]
        row_a[:, p * RS : (p + 1) * RS] = r["o2a"]
        row_b[:, p * RS : (p + 1) * RS] = r["o2b"]

    U1 = (col_a + row_a).T + b3
    U2 = (col_b + row_b).T + b1
    return np.concatenate([U1, U2], axis=1).astype(np.float32)


# revision 25
# speedup vs baseline: 1.3757x; 1.3757x over previous
"""FAME-GCN Trainium2 kernel, v3.

Computes, for merged adjacency final_A = temp + temp^T, temp = sum_k w_k A_k:
    U1 = final_A @ (feature @ W3) + b3
    U2 = final_A2 @ (feature @ W1) + b1
    out = concat(U1, U2, axis=1)          # [5000, 32]

Distribution: node rows sharded 625/core across 8 NeuronCores; the [16, N]
column-direction partials (temp^T S) are summed across cores on the host,
the row-direction results (temp S)[own rows] concatenate.

Host prep: w_k is folded into a bf16 cast of each adjacency (w_k * A_k),
so the on-device merge is a pure chain of 2x-mode bf16 tensor_adds and the
dir1 stationaries are plain support matrices.

Per core, 5 stripes of 125 rows:
  - the 12 scaled relation stripes arrive as [125, 5000] bf16 transfers
    spread over six DMA paths (sync-HWDGE, scalar-HWDGE, 4 SWDGE queues),
  - dir1 (temp^T S, both groups): per-relation matmuls against a stacked
    [125, 32] stationary (S3-half for A-relations, S1-half for A_t),
    PSUM-accumulated over all 12 relations, one flush per column block,
  - dir2 (temp S): bf16 tensor_add merge per group on DVE, then PE
    transposes (8 column chunks packed per PSUM bank, bulk-copied to SBUF
    by the scalar engine) feeding an S-stationary accumulation chain.
"""

import sys

if "/opt/trn_rl_repo" not in sys.path:
    sys.path.insert(0, "/opt/trn_rl_repo")

import ml_dtypes
import numpy as np

import concourse.bacc as bacc
import concourse.mybir as mybir
from concourse.tile import TileContext
from concourse.bass_utils import run_bass_kernel_spmd

F32 = mybir.dt.float32
BF16 = mybir.dt.bfloat16

N = 5000
NP = 5120  # padded row length for the gather path (row bytes % 256 == 0)
OUT = 16
K_A, K_AT = 3, 9
KTOT = K_A + K_AT
NCORES = 8
RS = N // NCORES  # 625 rows per core
STRIPE = 125
NSTRIPE = RS // STRIPE  # 5
CB = 512
NCB = (N + CB - 1) // CB  # 10
NJC = (N + 127) // 128  # 40
JPACK = 8  # transposed 128-col chunks packed per PSUM bank
HALF = NP // 2  # 2560: gather half-width (= 5 CB blocks)

_CACHE = {}


def _c_blocks():
    return [(cb * CB, min(CB, N - cb * CB)) for cb in range(NCB)]


def _j_chunks():
    return [(j, min(128, N - j * 128)) for j in range(NJC)]


def build():
    nc = bacc.Bacc(num_swdge_queues=4)

    adjh = nc.declare_dram_parameter("adjh", [K_A, RS, N], BF16, isOutput=False)
    adjg = nc.declare_dram_parameter("adjg", [K_AT, RS, NP], BF16, isOutput=False)
    idxs = nc.declare_dram_parameter("idxs", [128, 8 * NSTRIPE], mybir.dt.int16, isOutput=False)
    sst = nc.declare_dram_parameter("sst", [STRIPE, NSTRIPE * 64], BF16, isOutput=False)
    sfa = nc.declare_dram_parameter("sfa", [128, NJC * OUT], BF16, isOutput=False)
    sfb = nc.declare_dram_parameter("sfb", [128, NJC * OUT], BF16, isOutput=False)
    idt = nc.declare_dram_parameter("idt", [128, 128], BF16, isOutput=False)

    o1 = nc.declare_dram_parameter("o1", [32, N], F32, isOutput=True)
    o2a = nc.declare_dram_parameter("o2a", [OUT, RS], F32, isOutput=True)
    o2b = nc.declare_dram_parameter("o2b", [OUT, RS], F32, isOutput=True)

    # group-a relations (k<3) via sync-HWDGE; group-b via SWDGE gathers
    # spread over the 4 SWDGE queues

    with TileContext(nc) as tc:
        with (
            tc.tile_pool(name="persist", bufs=1) as pp,
            tc.tile_pool(name="raws", bufs=4) as rawsp,
            tc.tile_pool(name="rawg", bufs=20) as rawgp,
            tc.tile_pool(name="mrg", bufs=1) as mrgp,
            tc.tile_pool(name="strip", bufs=2) as stripp,
            tc.tile_pool(name="pdir", bufs=2, space="PSUM") as pdirp,
            tc.tile_pool(name="pt", bufs=2, space="PSUM") as ptp,
            tc.tile_pool(name="pd2", bufs=2, space="PSUM") as pd2p,
        ):
            # ---------------- persistent tiles ----------------
            sst_t = pp.tile([STRIPE, NSTRIPE * 64], BF16, tag="sst")
            nc.sync.dma_start(out=sst_t, in_=sst[:, :])
            sfa_t = pp.tile([128, NJC * OUT], BF16, tag="sfa")
            nc.sync.dma_start(out=sfa_t, in_=sfa[:, :])
            sfb_t = pp.tile([128, NJC * OUT], BF16, tag="sfb")
            nc.sync.dma_start(out=sfb_t, in_=sfb[:, :])
            id_t = pp.tile([128, 128], BF16, tag="idt")
            nc.sync.dma_start(out=id_t, in_=idt[:, :])
            ix = pp.tile([128, 8 * NSTRIPE], mybir.dt.int16, tag="ix")
            nc.sync.dma_start(out=ix, in_=idxs[:, :])

            o1sb = pp.tile([32, N], F32, tag="o1sb")
            acc2a = pp.tile([OUT, RS], F32, tag="acc2a")
            acc2b = pp.tile([OUT, RS], F32, tag="acc2b")

            for st in range(NSTRIPE):
                r0 = st * STRIPE
                # ---- loads: sync full rows for group a, half-row gathers
                # (aligned to CB blocks 0-4 / 5-9) for group b ----
                raw = {}
                for k in range(K_A):
                    t = rawsp.tile([STRIPE, N], BF16, tag="traw", name=f"t_{st}_{k}")
                    nc.sync.dma_start(out=t, in_=adjh[k, r0 : r0 + STRIPE, :])
                    raw[k] = t
                for h in range(2):
                    for k in range(K_A, KTOT):
                        t = rawgp.tile(
                            [128, 1, HALF], BF16, tag="trawg", name=f"t_{st}_{k}_{h}"
                        )
                        nc.gpsimd.dma_gather(
                            t,
                            adjg[k - K_A, :, h * HALF : (h + 1) * HALF],
                            ix[:, st * 8 : (st + 1) * 8],
                            128,
                            128,
                            HALF,
                            elem_step=NP,
                            queue_num=(2 * (k - K_A) + h) % 4,
                        )
                        raw[(k, h)] = t

                def rawsl(k, c0, cw):
                    if k < K_A:
                        return raw[k][:, c0 : c0 + cw]
                    h = 0 if c0 < HALF else 1
                    assert c0 + cw <= HALF or c0 >= HALF
                    return raw[(k, h)][:STRIPE, 0, c0 - h * HALF : c0 - h * HALF + cw]

                # ---- dir1: one stacked PSUM chain over all 12 relations ----
                sa = sst_t[:, st * 64 : st * 64 + 32]
                sb = sst_t[:, st * 64 + 32 : st * 64 + 64]
                for cb, (c0, cw) in enumerate(_c_blocks()):
                    pd = pdirp.tile([32, CB], F32, tag="pd", name=f"pd_{st}_{cb}")
                    for k in range(KTOT):
                        nc.tensor.matmul(
                            pd[:, :cw],
                            sa if k < K_A else sb,
                            rawsl(k, c0, cw),
                            start=(k == 0),
                            stop=(k == KTOT - 1),
                        )
                    dst = o1sb[:, c0 : c0 + cw]
                    if st == 0:
                        nc.vector.tensor_copy(out=dst, in_=pd[:, :cw])
                    else:
                        nc.vector.tensor_add(dst, dst, pd[:, :cw])

                # ---- merge per group: mrg = sum_k (w_k A_k) (bf16 adds) ----
                mrga = mrgp.tile([STRIPE, N], BF16, tag="mrga", name=f"mrga_{st}")
                mrgb = mrgp.tile([STRIPE, N], BF16, tag="mrgb", name=f"mrgb_{st}")
                nc.vector.tensor_add(mrga, rawsl(0, 0, N), rawsl(1, 0, N))
                nc.vector.tensor_add(mrga, mrga, rawsl(2, 0, N))
                for h, c0, cw in ((0, 0, HALF), (1, HALF, N - HALF)):
                    dst = mrgb[:, c0 : c0 + cw]
                    nc.vector.tensor_add(
                        dst, rawsl(3, c0, cw), rawsl(4, c0, cw)
                    )
                    for k in range(5, KTOT):
                        nc.vector.tensor_add(dst, dst, rawsl(k, c0, cw))

                # ---- dir2 per group: acc2[:, st] = (mrg @ S)^T ----
                for gname, mrg, sf_t, acc2 in (
                    ("a", mrga, sfa_t, acc2a),
                    ("b", mrgb, sfb_t, acc2b),
                ):
                    pd2 = pd2p.tile(
                        [OUT, 126], F32, tag="pd2", name=f"pd2_{st}_{gname}"
                    )
                    jcs = _j_chunks()
                    for jb in range(0, NJC, JPACK):
                        chunk = jcs[jb : jb + JPACK]
                        pt = ptp.tile(
                            [128, JPACK * 126],
                            BF16,
                            tag="pt",
                            name=f"pt_{st}_{gname}_{jb}",
                        )
                        for jj, (j, cjw) in enumerate(chunk):
                            nc.tensor.transpose(
                                pt[:cjw, jj * 126 : jj * 126 + 126],
                                mrg[:STRIPE, 128 * j : 128 * j + cjw],
                                id_t[:STRIPE, :126],
                            )
                        strip = stripp.tile(
                            [128, JPACK * 126],
                            BF16,
                            tag="tt",
                            name=f"tt_{st}_{gname}_{jb}",
                        )
                        wid = len(chunk) * 126
                        nc.scalar.copy(out=strip[:, :wid], in_=pt[:, :wid])
                        for jj, (j, cjw) in enumerate(chunk):
                            nc.tensor.matmul(
                                pd2[:, :126],
                                sf_t[:cjw, j * OUT : (j + 1) * OUT],
                                strip[:cjw, jj * 126 : jj * 126 + 126],
                                start=(j == 0),
                                stop=(j == NJC - 1),
                            )
                    nc.vector.tensor_copy(
                        out=acc2[:, r0 : r0 + STRIPE], in_=pd2[:, :STRIPE]
                    )

            nc.sync.dma_start(out=o1[:, :], in_=o1sb)
            nc.sync.dma_start(out=o2a[:, :], in_=acc2a)
            nc.sync.dma_start(out=o2b[:, :], in_=acc2b)

    nc.compile()
    return nc


def _make_inputs(feature, A, A_t, w2, wb, W3, W1):
    bf16 = ml_dtypes.bfloat16

    S3 = (feature @ W3).astype(np.float32)  # [N, 16]
    S1 = (feature @ W1).astype(np.float32)

    # sf: S laid out [128, NJC*16]; sf[p, j*16+o] = S[j*128+p, o]
    def make_sf(S):
        sf = np.zeros((128, NJC * OUT), dtype=np.float32)
        for j in range(NJC):
            w = min(128, N - j * 128)
            sf[:w, j * OUT : (j + 1) * OUT] = S[j * 128 : j * 128 + w]
        return sf.astype(bf16)

    sfa = make_sf(S3)
    sfb = make_sf(S1)
    eye = np.eye(128, dtype=bf16)

    # w_k folded into the adjacency cast
    Ascl = (A * w2[:, None, None]).astype(bf16)  # [3, N, N]
    Atscl = (A_t * wb[:, None, None]).astype(bf16)  # [9, N, N]

    idxs = np.full((128, 8 * NSTRIPE), -1, dtype=np.int16)
    for st in range(NSTRIPE):
        for j in range(STRIPE):
            for rep in range(8):
                idxs[j % 16 + 16 * rep, st * 8 + j // 16] = STRIPE * st + j

    in_maps = []
    for p in range(NCORES):
        r0 = p * RS
        adj = np.concatenate(
            [Ascl[:, r0 : r0 + RS, :], Atscl[:, r0 : r0 + RS, :]], axis=0
        )
        adjh = np.ascontiguousarray(adj[:K_A])  # [3, RS, N]
        adjg = np.zeros((K_AT, RS, NP), dtype=bf16)
        adjg[:, :, :N] = adj[K_A:]
        # stacked dir1 stationaries: [125, st*64 + (0:32 sa | 32:64 sb)]
        sst = np.zeros((STRIPE, NSTRIPE * 64), dtype=np.float32)
        for st in range(NSTRIPE):
            rows = slice(r0 + st * STRIPE, r0 + (st + 1) * STRIPE)
            sst[:, st * 64 : st * 64 + OUT] = S3[rows]
            sst[:, st * 64 + 48 : st * 64 + 64] = S1[rows]
        in_maps.append(
            {
                "adjh": adjh,
                "adjg": adjg,
                "idxs": idxs,
                "sst": sst.astype(bf16),
                "sfa": sfa,
                "sfb": sfb,
                "idt": eye,
            }
        )
    return in_maps


def kernel(feature, A, A_t, weight_b2, weight_b, W3, b3, W1, b1, **kw):
    feature = np.asarray(feature, dtype=np.float32)
    A = np.asarray(A, dtype=np.float32)
    A_t = np.asarray(A_t, dtype=np.float32)
    w2 = np.asarray(weight_b2, dtype=np.float32).reshape(K_A)
    wb = np.asarray(weight_b, dtype=np.float32).reshape(K_AT)
    W3 = np.asarray(W3, dtype=np.float32)
    W1 = np.asarray(W1, dtype=np.float32)
    b3 = np.asarray(b3, dtype=np.float32)
    b1 = np.asarray(b1, dtype=np.float32)

    if "nc" not in _CACHE:
        _CACHE["nc"] = build()
    nc = _CACHE["nc"]

    in_maps = _make_inputs(feature, A, A_t, w2, wb, W3, W1)
    _CACHE["in_maps"] = in_maps

    res = run_bass_kernel_spmd(nc, in_maps, core_ids=list(range(NCORES)))

    col_a = np.zeros((OUT, N), dtype=np.float32)
    col_b = np.zeros((OUT, N), dtype=np.float32)
    row_a = np.empty((OUT, N), dtype=np.float32)
    row_b = np.empty((OUT, N), dtype=np.float32)
    for p in range(NCORES):
        r = res.results[p]
        col_a += r["o1"][0:16]
        col_b += r["o1"][16:32]
        row_a[:, p * RS : (p + 1) * RS] = r["o2a"]
        row_b[:, p * RS : (p + 1) * RS] = r["o2b"]

    U1 = (col_a + row_a).T + b3
    U2 = (col_b + row_b).T + b1
    return np.concatenate([U1, U2], axis=1).astype(np.float32)


# revision 30
# speedup vs baseline: 1.3831x; 1.0054x over previous
"""FAME-GCN Trainium2 kernel, v3.

Computes, for merged adjacency final_A = temp + temp^T, temp = sum_k w_k A_k:
    U1 = final_A @ (feature @ W3) + b3
    U2 = final_A2 @ (feature @ W1) + b1
    out = concat(U1, U2, axis=1)          # [5000, 32]

Distribution: node rows sharded 625/core across 8 NeuronCores; the [16, N]
column-direction partials (temp^T S) are summed across cores on the host,
the row-direction results (temp S)[own rows] concatenate.

Host prep: w_k is folded into a bf16 cast of each adjacency (w_k * A_k),
so the on-device merge is a pure chain of 2x-mode bf16 tensor_adds and the
dir1 stationaries are plain support matrices.

Per core, 5 stripes of 125 rows:
  - the 12 scaled relation stripes arrive as [125, 5000] bf16 transfers
    spread over six DMA paths (sync-HWDGE, scalar-HWDGE, 4 SWDGE queues),
  - dir1 (temp^T S, both groups): per-relation matmuls against a stacked
    [125, 32] stationary (S3-half for A-relations, S1-half for A_t),
    PSUM-accumulated over all 12 relations, one flush per column block,
  - dir2 (temp S): bf16 tensor_add merge per group on DVE, then PE
    transposes (8 column chunks packed per PSUM bank, bulk-copied to SBUF
    by the scalar engine) feeding an S-stationary accumulation chain.
"""

import sys

if "/opt/trn_rl_repo" not in sys.path:
    sys.path.insert(0, "/opt/trn_rl_repo")

import ml_dtypes
import numpy as np

import concourse.bacc as bacc
import concourse.mybir as mybir
from concourse.tile import TileContext
from concourse.bass_utils import run_bass_kernel_spmd

F32 = mybir.dt.float32
BF16 = mybir.dt.bfloat16

N = 5000
NP = 5120  # padded row length for the gather path (row bytes % 256 == 0)
OUT = 16
K_A, K_AT = 3, 9
KTOT = K_A + K_AT
NCORES = 8
RS = N // NCORES  # 625 rows per core
STRIPE = 125
NSTRIPE = RS // STRIPE  # 5
CB = 512
NCB = (N + CB - 1) // CB  # 10
NJC = (N + 127) // 128  # 40
JPACK = 8  # transposed 128-col chunks packed per PSUM bank
HALF = NP // 2  # 2560: gather half-width (= 5 CB blocks)

_CACHE = {}


def _c_blocks():
    return [(cb * CB, min(CB, N - cb * CB)) for cb in range(NCB)]


def _j_chunks():
    return [(j, min(128, N - j * 128)) for j in range(NJC)]


def build():
    nc = bacc.Bacc(num_swdge_queues=4)

    adjh = nc.declare_dram_parameter(
        "adjh", [NSTRIPE, STRIPE, K_A * N], BF16, isOutput=False
    )
    adjg = nc.declare_dram_parameter("adjg", [K_AT, RS, NP], BF16, isOutput=False)
    idxs = nc.declare_dram_parameter("idxs", [128, 8 * NSTRIPE], mybir.dt.int16, isOutput=False)
    sst = nc.declare_dram_parameter("sst", [STRIPE, NSTRIPE * 64], BF16, isOutput=False)
    sfa = nc.declare_dram_parameter("sfa", [128, NJC * OUT], BF16, isOutput=False)
    sfb = nc.declare_dram_parameter("sfb", [128, NJC * OUT], BF16, isOutput=False)
    idt = nc.declare_dram_parameter("idt", [128, 128], BF16, isOutput=False)

    o1 = nc.declare_dram_parameter("o1", [32, N], F32, isOutput=True)
    o2a = nc.declare_dram_parameter("o2a", [OUT, RS], F32, isOutput=True)
    o2b = nc.declare_dram_parameter("o2b", [OUT, RS], F32, isOutput=True)

    # group-a relations (k<3) via sync-HWDGE; group-b via SWDGE gathers
    # spread over the 4 SWDGE queues

    with TileContext(nc) as tc:
        with (
            tc.tile_pool(name="persist", bufs=1) as pp,
            tc.tile_pool(name="raws", bufs=2) as rawsp,
            tc.tile_pool(name="rawg", bufs=18) as rawgp,
            tc.tile_pool(name="mrg", bufs=1) as mrgp,
            tc.tile_pool(name="strip", bufs=2) as stripp,
            tc.tile_pool(name="pdir", bufs=2, space="PSUM") as pdirp,
            tc.tile_pool(name="pt", bufs=2, space="PSUM") as ptp,
            tc.tile_pool(name="pd2", bufs=2, space="PSUM") as pd2p,
        ):
            # ---------------- persistent tiles ----------------
            sst_t = pp.tile([STRIPE, NSTRIPE * 64], BF16, tag="sst")
            nc.sync.dma_start(out=sst_t, in_=sst[:, :])
            sfa_t = pp.tile([128, NJC * OUT], BF16, tag="sfa")
            nc.sync.dma_start(out=sfa_t, in_=sfa[:, :])
            sfb_t = pp.tile([128, NJC * OUT], BF16, tag="sfb")
            nc.sync.dma_start(out=sfb_t, in_=sfb[:, :])
            id_t = pp.tile([128, 128], BF16, tag="idt")
            nc.sync.dma_start(out=id_t, in_=idt[:, :])
            ix = pp.tile([128, 8 * NSTRIPE], mybir.dt.int16, tag="ix")
            nc.sync.dma_start(out=ix, in_=idxs[:, :])

            o1sb = pp.tile([32, N], F32, tag="o1sb")
            acc2a = pp.tile([OUT, RS], F32, tag="acc2a")
            acc2b = pp.tile([OUT, RS], F32, tag="acc2b")

            for st in range(NSTRIPE):
                r0 = st * STRIPE
                # ---- loads: sync full rows for group a, half-row gathers
                # (aligned to CB blocks 0-4 / 5-9) for group b ----
                raw = {}
                tsync = rawsp.tile(
                    [STRIPE, K_A * N], BF16, tag="traw", name=f"ts_{st}"
                )
                nc.sync.dma_start(out=tsync, in_=adjh[st, :, :])
                for h in range(2):
                    for k in range(K_A, KTOT):
                        t = rawgp.tile(
                            [128, 1, HALF], BF16, tag="trawg", name=f"t_{st}_{k}_{h}"
                        )
                        nc.gpsimd.dma_gather(
                            t,
                            adjg[k - K_A, :, h * HALF : (h + 1) * HALF],
                            ix[:, st * 8 : (st + 1) * 8],
                            128,
                            128,
                            HALF,
                            elem_step=NP,
                            queue_num=(2 * (k - K_A) + h) % 4,
                        )
                        raw[(k, h)] = t

                def rawsl(k, c0, cw):
                    if k < K_A:
                        return tsync[:, k * N + c0 : k * N + c0 + cw]
                    h = 0 if c0 < HALF else 1
                    assert c0 + cw <= HALF or c0 >= HALF
                    return raw[(k, h)][:STRIPE, 0, c0 - h * HALF : c0 - h * HALF + cw]

                # ---- dir1: one stacked PSUM chain over all 12 relations ----
                sa = sst_t[:, st * 64 : st * 64 + 32]
                sb = sst_t[:, st * 64 + 32 : st * 64 + 64]
                for cb, (c0, cw) in enumerate(_c_blocks()):
                    pd = pdirp.tile([32, CB], F32, tag="pd", name=f"pd_{st}_{cb}")
                    for k in range(KTOT):
                        nc.tensor.matmul(
                            pd[:, :cw],
                            sa if k < K_A else sb,
                            rawsl(k, c0, cw),
                            start=(k == 0),
                            stop=(k == KTOT - 1),
                        )
                    dst = o1sb[:, c0 : c0 + cw]
                    if st == 0:
                        nc.vector.tensor_copy(out=dst, in_=pd[:, :cw])
                    else:
                        nc.vector.tensor_add(dst, dst, pd[:, :cw])

                # ---- merge per group: mrg = sum_k (w_k A_k) (bf16 adds) ----
                mrga = mrgp.tile([STRIPE, N], BF16, tag="mrga", name=f"mrga_{st}")
                mrgb = mrgp.tile([STRIPE, N], BF16, tag="mrgb", name=f"mrgb_{st}")
                nc.vector.tensor_add(mrga, rawsl(0, 0, N), rawsl(1, 0, N))
                nc.vector.tensor_add(mrga, mrga, rawsl(2, 0, N))
                for h, c0, cw in ((0, 0, HALF), (1, HALF, N - HALF)):
                    dst = mrgb[:, c0 : c0 + cw]
                    nc.vector.tensor_add(
                        dst, rawsl(3, c0, cw), rawsl(4, c0, cw)
                    )
                    for k in range(5, KTOT):
                        nc.vector.tensor_add(dst, dst, rawsl(k, c0, cw))

                # ---- dir2 per group: acc2[:, st] = (mrg @ S)^T ----
                for gname, mrg, sf_t, acc2 in (
                    ("a", mrga, sfa_t, acc2a),
                    ("b", mrgb, sfb_t, acc2b),
                ):
                    pd2 = pd2p.tile(
                        [OUT, 126], F32, tag="pd2", name=f"pd2_{st}_{gname}"
                    )
                    jcs = _j_chunks()
                    for jb in range(0, NJC, JPACK):
                        chunk = jcs[jb : jb + JPACK]
                        pt = ptp.tile(
                            [128, JPACK * 126],
                            BF16,
                            tag="pt",
                            name=f"pt_{st}_{gname}_{jb}",
                        )
                        for jj, (j, cjw) in enumerate(chunk):
                            nc.tensor.transpose(
                                pt[:cjw, jj * 126 : jj * 126 + 126],
                                mrg[:STRIPE, 128 * j : 128 * j + cjw],
                                id_t[:STRIPE, :126],
                            )
                        strip = stripp.tile(
                            [128, JPACK * 126],
                            BF16,
                            tag="tt",
                            name=f"tt_{st}_{gname}_{jb}",
                        )
                        wid = len(chunk) * 126
                        nc.scalar.copy(out=strip[:, :wid], in_=pt[:, :wid])
                        for jj, (j, cjw) in enumerate(chunk):
                            nc.tensor.matmul(
                                pd2[:, :126],
                                sf_t[:cjw, j * OUT : (j + 1) * OUT],
                                strip[:cjw, jj * 126 : jj * 126 + 126],
                                start=(j == 0),
                                stop=(j == NJC - 1),
                            )
                    nc.vector.tensor_copy(
                        out=acc2[:, r0 : r0 + STRIPE], in_=pd2[:, :STRIPE]
                    )

            nc.sync.dma_start(out=o1[:, :], in_=o1sb)
            nc.sync.dma_start(out=o2a[:, :], in_=acc2a)
            nc.sync.dma_start(out=o2b[:, :], in_=acc2b)

    nc.compile()
    return nc


def _make_inputs(feature, A, A_t, w2, wb, W3, W1):
    bf16 = ml_dtypes.bfloat16

    S3 = (feature @ W3).astype(np.float32)  # [N, 16]
    S1 = (feature @ W1).astype(np.float32)

    # sf: S laid out [128, NJC*16]; sf[p, j*16+o] = S[j*128+p, o]
    def make_sf(S):
        sf = np.zeros((128, NJC * OUT), dtype=np.float32)
        for j in range(NJC):
            w = min(128, N - j * 128)
            sf[:w, j * OUT : (j + 1) * OUT] = S[j * 128 : j * 128 + w]
        return sf.astype(bf16)

    sfa = make_sf(S3)
    sfb = make_sf(S1)
    eye = np.eye(128, dtype=bf16)

    # w_k folded into the adjacency cast
    Ascl = (A * w2[:, None, None]).astype(bf16)  # [3, N, N]
    Atscl = (A_t * wb[:, None, None]).astype(bf16)  # [9, N, N]

    idxs = np.full((128, 8 * NSTRIPE), -1, dtype=np.int16)
    for st in range(NSTRIPE):
        for j in range(STRIPE):
            for rep in range(8):
                idxs[j % 16 + 16 * rep, st * 8 + j // 16] = STRIPE * st + j

    in_maps = []
    for p in range(NCORES):
        r0 = p * RS
        adj = np.concatenate(
            [Ascl[:, r0 : r0 + RS, :], Atscl[:, r0 : r0 + RS, :]], axis=0
        )
        # group-a relations interleaved per stripe: adjh[st, r, k*N:(k+1)*N]
        adjh = np.ascontiguousarray(
            adj[:K_A].reshape(K_A, NSTRIPE, STRIPE, N).transpose(1, 2, 0, 3)
        ).reshape(NSTRIPE, STRIPE, K_A * N)
        adjg = np.zeros((K_AT, RS, NP), dtype=bf16)
        adjg[:, :, :N] = adj[K_A:]
        # stacked dir1 stationaries: [125, st*64 + (0:32 sa | 32:64 sb)]
        sst = np.zeros((STRIPE, NSTRIPE * 64), dtype=np.float32)
        for st in range(NSTRIPE):
            rows = slice(r0 + st * STRIPE, r0 + (st + 1) * STRIPE)
            sst[:, st * 64 : st * 64 + OUT] = S3[rows]
            sst[:, st * 64 + 48 : st * 64 + 64] = S1[rows]
        in_maps.append(
            {
                "adjh": adjh,
                "adjg": adjg,
                "idxs": idxs,
                "sst": sst.astype(bf16),
                "sfa": sfa,
                "sfb": sfb,
                "idt": eye,
            }
        )
    return in_maps


def kernel(feature, A, A_t, weight_b2, weight_b, W3, b3, W1, b1, **kw):
    feature = np.asarray(feature, dtype=np.float32)
    A = np.asarray(A, dtype=np.float32)
    A_t = np.asarray(A_t, dtype=np.float32)
    w2 = np.asarray(weight_b2, dtype=np.float32).reshape(K_A)
    wb = np.asarray(weight_b, dtype=np.float32).reshape(K_AT)
    W3 = np.asarray(W3, dtype=np.float32)
    W1 = np.asarray(W1, dtype=np.float32)
    b3 = np.asarray(b3, dtype=np.float32)
    b1 = np.asarray(b1, dtype=np.float32)

    if "nc" not in _CACHE:
        _CACHE["nc"] = build()
    nc = _CACHE["nc"]

    in_maps = _make_inputs(feature, A, A_t, w2, wb, W3, W1)
    _CACHE["in_maps"] = in_maps

    res = run_bass_kernel_spmd(nc, in_maps, core_ids=list(range(NCORES)))

    col_a = np.zeros((OUT, N), dtype=np.float32)
    col_b = np.zeros((OUT, N), dtype=np.float32)
    row_a = np.empty((OUT, N), dtype=np.float32)
    row_b = np.empty((OUT, N), dtype=np.float32)
    for p in range(NCORES):
        r = res.results[p]
        col_a += r["o1"][0:16]
        col_b += r["o1"][16:32]
        row_a[:, p * RS : (p + 1) * RS] = r["o2a"]
        row_b[:, p * RS : (p + 1) * RS] = r["o2b"]

    U1 = (col_a + row_a).T + b3
    U2 = (col_b + row_b).T + b1
    return np.concatenate([U1, U2], axis=1).astype(np.float32)


# revision 38
# speedup vs baseline: 1.3865x; 1.0025x over previous
"""FAME-GCN Trainium2 kernel, v3.

Computes, for merged adjacency final_A = temp + temp^T, temp = sum_k w_k A_k:
    U1 = final_A @ (feature @ W3) + b3
    U2 = final_A2 @ (feature @ W1) + b1
    out = concat(U1, U2, axis=1)          # [5000, 32]

Distribution: node rows sharded 625/core across 8 NeuronCores; the [16, N]
column-direction partials (temp^T S) are summed across cores on the host,
the row-direction results (temp S)[own rows] concatenate.

Host prep: w_k is folded into a bf16 cast of each adjacency (w_k * A_k),
so the on-device merge is a pure chain of 2x-mode bf16 tensor_adds and the
dir1 stationaries are plain support matrices.

Per core, 5 stripes of 125 rows:
  - the 12 scaled relation stripes arrive as [125, 5000] bf16 transfers
    spread over six DMA paths (sync-HWDGE, scalar-HWDGE, 4 SWDGE queues),
  - dir1 (temp^T S, both groups): per-relation matmuls against a stacked
    [125, 32] stationary (S3-half for A-relations, S1-half for A_t),
    PSUM-accumulated over all 12 relations, one flush per column block,
  - dir2 (temp S): bf16 tensor_add merge per group on DVE, then PE
    transposes (8 column chunks packed per PSUM bank, bulk-copied to SBUF
    by the scalar engine) feeding an S-stationary accumulation chain.
"""

import sys

if "/opt/trn_rl_repo" not in sys.path:
    sys.path.insert(0, "/opt/trn_rl_repo")

import ml_dtypes
import numpy as np

import concourse.bacc as bacc
import concourse.mybir as mybir
from concourse.tile import TileContext
from concourse.bass_utils import run_bass_kernel_spmd

F32 = mybir.dt.float32
BF16 = mybir.dt.bfloat16

N = 5000
NP = 5120  # padded row length for the gather path (row bytes % 256 == 0)
OUT = 16
K_A, K_AT = 3, 9
KTOT = K_A + K_AT
NCORES = 8
RS = N // NCORES  # 625 rows per core
STRIPE = 125
NSTRIPE = RS // STRIPE  # 5
CB = 512
NCB = (N + CB - 1) // CB  # 10
NJC = (N + 127) // 128  # 40
JPACK = 8  # transposed 128-col chunks packed per PSUM bank
HALF = NP // 2  # 2560: gather half-width (= 5 CB blocks)

_CACHE = {}


def _c_blocks():
    return [(cb * CB, min(CB, N - cb * CB)) for cb in range(NCB)]


def _j_chunks():
    return [(j, min(128, N - j * 128)) for j in range(NJC)]


def build():
    nc = bacc.Bacc(num_swdge_queues=4)

    adjh = nc.declare_dram_parameter(
        "adjh", [NSTRIPE, STRIPE, K_A * N], BF16, isOutput=False
    )
    adjg = nc.declare_dram_parameter("adjg", [K_AT, RS, NP], BF16, isOutput=False)
    idxs = nc.declare_dram_parameter("idxs", [128, 8 * NSTRIPE], mybir.dt.int16, isOutput=False)
    sst = nc.declare_dram_parameter("sst", [STRIPE, NSTRIPE * 32], BF16, isOutput=False)
    sfa = nc.declare_dram_parameter("sfa", [128, NJC * OUT], BF16, isOutput=False)
    sfb = nc.declare_dram_parameter("sfb", [128, NJC * OUT], BF16, isOutput=False)
    idt = nc.declare_dram_parameter("idt", [128, 128], BF16, isOutput=False)

    o1 = nc.declare_dram_parameter("o1", [48, N], F32, isOutput=True)
    o2a = nc.declare_dram_parameter("o2a", [OUT, RS], F32, isOutput=True)
    o2b = nc.declare_dram_parameter("o2b", [OUT, RS], F32, isOutput=True)

    # group-a relations (k<3) via sync-HWDGE; group-b via SWDGE gathers
    # spread over the 4 SWDGE queues

    with TileContext(nc) as tc:
        with (
            tc.tile_pool(name="persist", bufs=1) as pp,
            tc.tile_pool(name="raws", bufs=2) as rawsp,
            tc.tile_pool(name="rawg", bufs=18) as rawgp,
            tc.tile_pool(name="mrg", bufs=1) as mrgp,
            tc.tile_pool(name="strip", bufs=2) as stripp,
            tc.tile_pool(name="pdir", bufs=2, space="PSUM") as pdirp,
            tc.tile_pool(name="pt", bufs=2, space="PSUM") as ptp,
            tc.tile_pool(name="pd2", bufs=2, space="PSUM") as pd2p,
        ):
            # ---------------- persistent tiles ----------------
            sst_t = pp.tile([STRIPE, NSTRIPE * 32], BF16, tag="sst")
            nc.sync.dma_start(out=sst_t, in_=sst[:, :])
            sfa_t = pp.tile([128, NJC * OUT], BF16, tag="sfa")
            nc.sync.dma_start(out=sfa_t, in_=sfa[:, :])
            sfb_t = pp.tile([128, NJC * OUT], BF16, tag="sfb")
            nc.sync.dma_start(out=sfb_t, in_=sfb[:, :])
            id_t = pp.tile([128, 128], BF16, tag="idt")
            nc.sync.dma_start(out=id_t, in_=idt[:, :])
            ix = pp.tile([128, 8 * NSTRIPE], mybir.dt.int16, tag="ix")
            nc.sync.dma_start(out=ix, in_=idxs[:, :])

            o1sb = pp.tile([48, N], F32, tag="o1sb")
            acc2a = pp.tile([OUT, RS], F32, tag="acc2a")
            acc2b = pp.tile([OUT, RS], F32, tag="acc2b")

            for st in range(NSTRIPE):
                r0 = st * STRIPE
                # ---- loads: sync full rows for group a, half-row gathers
                # (aligned to CB blocks 0-4 / 5-9) for group b ----
                raw = {}
                tsync = rawsp.tile(
                    [STRIPE, K_A * N], BF16, tag="traw", name=f"ts_{st}"
                )
                nc.sync.dma_start(out=tsync, in_=adjh[st, :, :])
                for h in range(2):
                    for k in range(K_A, KTOT):
                        t = rawgp.tile(
                            [128, 1, HALF], BF16, tag="trawg", name=f"t_{st}_{k}_{h}"
                        )
                        nc.gpsimd.dma_gather(
                            t,
                            adjg[k - K_A, :, h * HALF : (h + 1) * HALF],
                            ix[:, st * 8 : (st + 1) * 8],
                            128,
                            128,
                            HALF,
                            elem_step=NP,
                            queue_num=(2 * (k - K_A) + h) % 4,
                        )
                        raw[(k, h)] = t

                def rawsl(k, c0, cw):
                    if k < K_A:
                        return tsync[:, k * N + c0 : k * N + c0 + cw]
                    h = 0 if c0 < HALF else 1
                    assert c0 + cw <= HALF or c0 >= HALF
                    return raw[(k, h)][:STRIPE, 0, c0 - h * HALF : c0 - h * HALF + cw]

                # ---- dir1: two independent PSUM chains (group a from the
                # sync tile, group b from gathers) into disjoint partition
                # ranges of one bank; a completes early, freeing tsync ----
                sa = sst_t[:, st * 32 : st * 32 + OUT]
                sb = sst_t[:, st * 32 + OUT : st * 32 + 32]
                for cb, (c0, cw) in enumerate(_c_blocks()):
                    pd = pdirp.tile([48, CB], F32, tag="pd", name=f"pd_{st}_{cb}")
                    for k in range(K_A):
                        nc.tensor.matmul(
                            pd[0:OUT, :cw],
                            sa,
                            rawsl(k, c0, cw),
                            start=(k == 0),
                            stop=(k == K_A - 1),
                        )
                    for k in range(K_A, KTOT):
                        nc.tensor.matmul(
                            pd[32:48, :cw],
                            sb,
                            rawsl(k, c0, cw),
                            start=(k == K_A),
                            stop=(k == KTOT - 1),
                        )
                    dst = o1sb[:, c0 : c0 + cw]
                    if st == 0:
                        nc.vector.tensor_copy(out=dst, in_=pd[:, :cw])
                    else:
                        nc.vector.tensor_add(dst, dst, pd[:, :cw])

                # ---- merge per group: mrg = sum_k (w_k A_k) (bf16 adds) ----
                mrga = mrgp.tile([STRIPE, N], BF16, tag="mrga", name=f"mrga_{st}")
                mrgb = mrgp.tile([STRIPE, N], BF16, tag="mrgb", name=f"mrgb_{st}")
                nc.vector.tensor_add(mrga, rawsl(0, 0, N), rawsl(1, 0, N))
                nc.vector.tensor_add(mrga, mrga, rawsl(2, 0, N))
                for h, c0, cw in ((0, 0, HALF), (1, HALF, N - HALF)):
                    dst = mrgb[:, c0 : c0 + cw]
                    nc.vector.tensor_add(
                        dst, rawsl(3, c0, cw), rawsl(4, c0, cw)
                    )
                    for k in range(5, KTOT):
                        nc.vector.tensor_add(dst, dst, rawsl(k, c0, cw))

                # ---- dir2 per group: acc2[:, st] = (mrg @ S)^T ----
                for gname, mrg, sf_t, acc2 in (
                    ("a", mrga, sfa_t, acc2a),
                    ("b", mrgb, sfb_t, acc2b),
                ):
                    pd2 = pd2p.tile(
                        [OUT, 126], F32, tag="pd2", name=f"pd2_{st}_{gname}"
                    )
                    jcs = _j_chunks()
                    for jb in range(0, NJC, JPACK):
                        chunk = jcs[jb : jb + JPACK]
                        pt = ptp.tile(
                            [128, JPACK * 126],
                            BF16,
                            tag="pt",
                            name=f"pt_{st}_{gname}_{jb}",
                        )
                        for jj, (j, cjw) in enumerate(chunk):
                            nc.tensor.transpose(
                                pt[:cjw, jj * 126 : jj * 126 + 126],
                                mrg[:STRIPE, 128 * j : 128 * j + cjw],
                                id_t[:STRIPE, :126],
                            )
                        strip = stripp.tile(
                            [128, JPACK * 126],
                            BF16,
                            tag="tt",
                            name=f"tt_{st}_{gname}_{jb}",
                        )
                        wid = len(chunk) * 126
                        nc.scalar.copy(out=strip[:, :wid], in_=pt[:, :wid])
                        for jj, (j, cjw) in enumerate(chunk):
                            nc.tensor.matmul(
                                pd2[:, :126],
                                sf_t[:cjw, j * OUT : (j + 1) * OUT],
                                strip[:cjw, jj * 126 : jj * 126 + 126],
                                start=(j == 0),
                                stop=(j == NJC - 1),
                            )
                    nc.vector.tensor_copy(
                        out=acc2[:, r0 : r0 + STRIPE], in_=pd2[:, :STRIPE]
                    )

            nc.sync.dma_start(out=o1[:, :], in_=o1sb)
            nc.sync.dma_start(out=o2a[:, :], in_=acc2a)
            nc.sync.dma_start(out=o2b[:, :], in_=acc2b)

    nc.compile()
    return nc


def _make_inputs(feature, A, A_t, w2, wb, W3, W1):
    bf16 = ml_dtypes.bfloat16

    S3 = (feature @ W3).astype(np.float32)  # [N, 16]
    S1 = (feature @ W1).astype(np.float32)

    # sf: S laid out [128, NJC*16]; sf[p, j*16+o] = S[j*128+p, o]
    def make_sf(S):
        sf = np.zeros((128, NJC * OUT), dtype=np.float32)
        for j in range(NJC):
            w = min(128, N - j * 128)
            sf[:w, j * OUT : (j + 1) * OUT] = S[j * 128 : j * 128 + w]
        return sf.astype(bf16)

    sfa = make_sf(S3)
    sfb = make_sf(S1)
    eye = np.eye(128, dtype=bf16)

    # w_k folded into the adjacency cast
    Ascl = (A * w2[:, None, None]).astype(bf16)  # [3, N, N]
    Atscl = (A_t * wb[:, None, None]).astype(bf16)  # [9, N, N]

    idxs = np.full((128, 8 * NSTRIPE), -1, dtype=np.int16)
    for st in range(NSTRIPE):
        for j in range(STRIPE):
            for rep in range(8):
                idxs[j % 16 + 16 * rep, st * 8 + j // 16] = STRIPE * st + j

    in_maps = []
    for p in range(NCORES):
        r0 = p * RS
        adj = np.concatenate(
            [Ascl[:, r0 : r0 + RS, :], Atscl[:, r0 : r0 + RS, :]], axis=0
        )
        # group-a relations interleaved per stripe: adjh[st, r, k*N:(k+1)*N]
        adjh = np.ascontiguousarray(
            adj[:K_A].reshape(K_A, NSTRIPE, STRIPE, N).transpose(1, 2, 0, 3)
        ).reshape(NSTRIPE, STRIPE, K_A * N)
        adjg = np.zeros((K_AT, RS, NP), dtype=bf16)
        adjg[:, :, :N] = adj[K_A:]
        # dir1 stationaries: [125, st*32 + (0:16 S3 | 16:32 S1)]
        sst = np.zeros((STRIPE, NSTRIPE * 32), dtype=np.float32)
        for st in range(NSTRIPE):
            rows = slice(r0 + st * STRIPE, r0 + (st + 1) * STRIPE)
            sst[:, st * 32 : st * 32 + OUT] = S3[rows]
            sst[:, st * 32 + OUT : st * 32 + 32] = S1[rows]
        in_maps.append(
            {
                "adjh": adjh,
                "adjg": adjg,
                "idxs": idxs,
                "sst": sst.astype(bf16),
                "sfa": sfa,
                "sfb": sfb,
                "idt": eye,
            }
        )
    return in_maps


def kernel(feature, A, A_t, weight_b2, weight_b, W3, b3, W1, b1, **kw):
    feature = np.asarray(feature, dtype=np.float32)
    A = np.asarray(A, dtype=np.float32)
    A_t = np.asarray(A_t, dtype=np.float32)
    w2 = np.asarray(weight_b2, dtype=np.float32).reshape(K_A)
    wb = np.asarray(weight_b, dtype=np.float32).reshape(K_AT)
    W3 = np.asarray(W3, dtype=np.float32)
    W1 = np.asarray(W1, dtype=np.float32)
    b3 = np.asarray(b3, dtype=np.float32)
    b1 = np.asarray(b1, dtype=np.float32)

    if "nc" not in _CACHE:
        _CACHE["nc"] = build()
    nc = _CACHE["nc"]

    in_maps = _make_inputs(feature, A, A_t, w2, wb, W3, W1)
    _CACHE["in_maps"] = in_maps

    res = run_bass_kernel_spmd(nc, in_maps, core_ids=list(range(NCORES)))

    col_a = np.zeros((OUT, N), dtype=np.float32)
    col_b = np.zeros((OUT, N), dtype=np.float32)
    row_a = np.empty((OUT, N), dtype=np.float32)
    row_b = np.empty((OUT, N), dtype=np.float32)
    for p in range(NCORES):
        r = res.results[p]
        col_a += r["o1"][0:16]
        col_b += r["o1"][32:48]
        row_a[:, p * RS : (p + 1) * RS] = r["o2a"]
        row_b[:, p * RS : (p + 1) * RS] = r["o2b"]

    U1 = (col_a + row_a).T + b3
    U2 = (col_b + row_b).T + b1
    return np.concatenate([U1, U2], axis=1).astype(np.float32)


# revision 43
# speedup vs baseline: 1.7841x; 1.2867x over previous
"""FAME-GCN Trainium2 kernel, v3.

Computes, for merged adjacency final_A = temp + temp^T, temp = sum_k w_k A_k:
    U1 = final_A @ (feature @ W3) + b3
    U2 = final_A2 @ (feature @ W1) + b1
    out = concat(U1, U2, axis=1)          # [5000, 32]

Distribution: node rows sharded 625/core across 8 NeuronCores; the [16, N]
column-direction partials (temp^T S) are summed across cores on the host,
the row-direction results (temp S)[own rows] concatenate.

Host prep: w_k is folded into a bf16 cast of each adjacency (w_k * A_k),
so the on-device merge is a pure chain of 2x-mode bf16 tensor_adds and the
dir1 stationaries are plain support matrices.

Per core, 5 stripes of 125 rows:
  - the 12 scaled relation stripes arrive as [125, 5000] bf16 transfers
    spread over six DMA paths (sync-HWDGE, scalar-HWDGE, 4 SWDGE queues),
  - dir1 (temp^T S, both groups): per-relation matmuls against a stacked
    [125, 32] stationary (S3-half for A-relations, S1-half for A_t),
    PSUM-accumulated over all 12 relations, one flush per column block,
  - dir2 (temp S): bf16 tensor_add merge per group on DVE, then PE
    transposes (8 column chunks packed per PSUM bank, bulk-copied to SBUF
    by the scalar engine) feeding an S-stationary accumulation chain.
"""

import sys

if "/opt/trn_rl_repo" not in sys.path:
    sys.path.insert(0, "/opt/trn_rl_repo")

import ml_dtypes
import numpy as np

import concourse.bacc as bacc
import concourse.mybir as mybir
from concourse.tile import TileContext
from concourse.bass_utils import run_bass_kernel_spmd

F32 = mybir.dt.float32
BF16 = mybir.dt.bfloat16

N = 5000
NP = 5120  # padded row length for the gather path (row bytes % 256 == 0)
OUT = 16
K_A, K_AT = 3, 9
KTOT = K_A + K_AT
NCORES = 8
RS = N // NCORES  # 625 rows per core
STRIPE = 125
NSTRIPE = RS // STRIPE  # 5
CB = 512
NCB = (N + CB - 1) // CB  # 10
NJC = (N + 127) // 128  # 40
JPACK = 8  # transposed 128-col chunks packed per PSUM bank
HALF = NP // 2  # 2560: gather half-width (= 5 CB blocks)

_CACHE = {}


def _c_blocks():
    return [(cb * CB, min(CB, N - cb * CB)) for cb in range(NCB)]


def _j_chunks():
    return [(j, min(128, N - j * 128)) for j in range(NJC)]


def build():
    nc = bacc.Bacc(num_swdge_queues=4)

    adjg = nc.declare_dram_parameter("adjg", [KTOT, RS, NP], BF16, isOutput=False)
    idxs = nc.declare_dram_parameter("idxs", [128, 8 * NSTRIPE], mybir.dt.int16, isOutput=False)
    sst = nc.declare_dram_parameter("sst", [STRIPE, NSTRIPE * 32], BF16, isOutput=False)
    sfa = nc.declare_dram_parameter("sfa", [128, NJC * OUT], BF16, isOutput=False)
    sfb = nc.declare_dram_parameter("sfb", [128, NJC * OUT], BF16, isOutput=False)
    idt = nc.declare_dram_parameter("idt", [128, 128], BF16, isOutput=False)

    o1 = nc.declare_dram_parameter("o1", [48, N], F32, isOutput=True)
    o2a = nc.declare_dram_parameter("o2a", [OUT, RS], F32, isOutput=True)
    o2b = nc.declare_dram_parameter("o2b", [OUT, RS], F32, isOutput=True)

    # group-a relations (k<3) via sync-HWDGE; group-b via SWDGE gathers
    # spread over the 4 SWDGE queues

    with TileContext(nc) as tc:
        with (
            tc.tile_pool(name="persist", bufs=1) as pp,
            tc.tile_pool(name="rawa", bufs=4) as rawap,
            tc.tile_pool(name="rawg", bufs=18) as rawgp,
            tc.tile_pool(name="mrg", bufs=1) as mrgp,
            tc.tile_pool(name="strip", bufs=2) as stripp,
            tc.tile_pool(name="pdir", bufs=2, space="PSUM") as pdirp,
            tc.tile_pool(name="pt", bufs=2, space="PSUM") as ptp,
            tc.tile_pool(name="pd2", bufs=2, space="PSUM") as pd2p,
        ):
            # ---------------- persistent tiles ----------------
            sst_t = pp.tile([STRIPE, NSTRIPE * 32], BF16, tag="sst")
            nc.sync.dma_start(out=sst_t, in_=sst[:, :])
            sfa_t = pp.tile([128, NJC * OUT], BF16, tag="sfa")
            nc.sync.dma_start(out=sfa_t, in_=sfa[:, :])
            sfb_t = pp.tile([128, NJC * OUT], BF16, tag="sfb")
            nc.sync.dma_start(out=sfb_t, in_=sfb[:, :])
            id_t = pp.tile([128, 128], BF16, tag="idt")
            nc.sync.dma_start(out=id_t, in_=idt[:, :])
            ix = pp.tile([128, 8 * NSTRIPE], mybir.dt.int16, tag="ix")
            nc.sync.dma_start(out=ix, in_=idxs[:, :])

            o1sb = pp.tile([48, N], F32, tag="o1sb")
            acc2a = pp.tile([OUT, RS], F32, tag="acc2a")
            acc2b = pp.tile([OUT, RS], F32, tag="acc2b")

            for st in range(NSTRIPE):
                r0 = st * STRIPE
                # ---- loads: full-row gathers for group a, half-row gathers
                # (aligned to CB blocks 0-4 / 5-9) for group b ----
                raw = {}
                qn = st  # rotate queue assignment across stripes
                for k in range(K_A):
                    t = rawap.tile(
                        [128, 1, NP], BF16, tag="trawa", name=f"t_{st}_{k}"
                    )
                    nc.gpsimd.dma_gather(
                        t,
                        adjg[k, :, :],
                        ix[:, st * 8 : (st + 1) * 8],
                        128,
                        128,
                        NP,
                        elem_step=NP,
                        queue_num=qn % 4,
                    )
                    qn += 1
                    raw[k] = t
                for h in range(2):
                    for k in range(K_A, KTOT):
                        t = rawgp.tile(
                            [128, 1, HALF], BF16, tag="trawg", name=f"t_{st}_{k}_{h}"
                        )
                        nc.gpsimd.dma_gather(
                            t,
                            adjg[k, :, h * HALF : (h + 1) * HALF],
                            ix[:, st * 8 : (st + 1) * 8],
                            128,
                            128,
                            HALF,
                            elem_step=NP,
                            queue_num=qn % 4,
                        )
                        qn += 1
                        raw[(k, h)] = t

                def rawsl(k, c0, cw):
                    if k < K_A:
                        return raw[k][:STRIPE, 0, c0 : c0 + cw]
                    h = 0 if c0 < HALF else 1
                    assert c0 + cw <= HALF or c0 >= HALF
                    return raw[(k, h)][:STRIPE, 0, c0 - h * HALF : c0 - h * HALF + cw]

                # ---- dir1: two independent PSUM chains (group a from the
                # sync tile, group b from gathers) into disjoint partition
                # ranges of one bank; a completes early, freeing tsync ----
                sa = sst_t[:, st * 32 : st * 32 + OUT]
                sb = sst_t[:, st * 32 + OUT : st * 32 + 32]
                for cb, (c0, cw) in enumerate(_c_blocks()):
                    pd = pdirp.tile([48, CB], F32, tag="pd", name=f"pd_{st}_{cb}")
                    for k in range(K_A):
                        nc.tensor.matmul(
                            pd[0:OUT, :cw],
                            sa,
                            rawsl(k, c0, cw),
                            start=(k == 0),
                            stop=(k == K_A - 1),
                        )
                    for k in range(K_A, KTOT):
                        nc.tensor.matmul(
                            pd[32:48, :cw],
                            sb,
                            rawsl(k, c0, cw),
                            start=(k == K_A),
                            stop=(k == KTOT - 1),
                        )
                    dst = o1sb[:, c0 : c0 + cw]
                    if st == 0:
                        nc.vector.tensor_copy(out=dst, in_=pd[:, :cw])
                    else:
                        nc.vector.tensor_add(dst, dst, pd[:, :cw])

                # ---- merge per group: mrg = sum_k (w_k A_k) (bf16 adds) ----
                mrga = mrgp.tile([STRIPE, N], BF16, tag="mrga", name=f"mrga_{st}")
                mrgb = mrgp.tile([STRIPE, N], BF16, tag="mrgb", name=f"mrgb_{st}")
                nc.vector.tensor_add(mrga, rawsl(0, 0, N), rawsl(1, 0, N))
                nc.vector.tensor_add(mrga, mrga, rawsl(2, 0, N))
                for h, c0, cw in ((0, 0, HALF), (1, HALF, N - HALF)):
                    dst = mrgb[:, c0 : c0 + cw]
                    nc.vector.tensor_add(
                        dst, rawsl(3, c0, cw), rawsl(4, c0, cw)
                    )
                    for k in range(5, KTOT):
                        nc.vector.tensor_add(dst, dst, rawsl(k, c0, cw))

                # ---- dir2 per group: acc2[:, st] = (mrg @ S)^T ----
                for gname, mrg, sf_t, acc2 in (
                    ("a", mrga, sfa_t, acc2a),
                    ("b", mrgb, sfb_t, acc2b),
                ):
                    pd2 = pd2p.tile(
                        [OUT, 126], F32, tag="pd2", name=f"pd2_{st}_{gname}"
                    )
                    jcs = _j_chunks()
                    for jb in range(0, NJC, JPACK):
                        chunk = jcs[jb : jb + JPACK]
                        pt = ptp.tile(
                            [128, JPACK * 126],
                            BF16,
                            tag="pt",
                            name=f"pt_{st}_{gname}_{jb}",
                        )
                        for jj, (j, cjw) in enumerate(chunk):
                            nc.tensor.transpose(
                                pt[:cjw, jj * 126 : jj * 126 + 126],
                                mrg[:STRIPE, 128 * j : 128 * j + cjw],
                                id_t[:STRIPE, :126],
                            )
                        strip = stripp.tile(
                            [128, JPACK * 126],
                            BF16,
                            tag="tt",
                            name=f"tt_{st}_{gname}_{jb}",
                        )
                        wid = len(chunk) * 126
                        nc.scalar.copy(out=strip[:, :wid], in_=pt[:, :wid])
                        for jj, (j, cjw) in enumerate(chunk):
                            nc.tensor.matmul(
                                pd2[:, :126],
                                sf_t[:cjw, j * OUT : (j + 1) * OUT],
                                strip[:cjw, jj * 126 : jj * 126 + 126],
                                start=(j == 0),
                                stop=(j == NJC - 1),
                            )
                    nc.vector.tensor_copy(
                        out=acc2[:, r0 : r0 + STRIPE], in_=pd2[:, :STRIPE]
                    )

            nc.sync.dma_start(out=o1[:, :], in_=o1sb)
            nc.sync.dma_start(out=o2a[:, :], in_=acc2a)
            nc.sync.dma_start(out=o2b[:, :], in_=acc2b)

    nc.compile()
    return nc


def _make_inputs(feature, A, A_t, w2, wb, W3, W1):
    bf16 = ml_dtypes.bfloat16

    S3 = (feature @ W3).astype(np.float32)  # [N, 16]
    S1 = (feature @ W1).astype(np.float32)

    # sf: S laid out [128, NJC*16]; sf[p, j*16+o] = S[j*128+p, o]
    def make_sf(S):
        sf = np.zeros((128, NJC * OUT), dtype=np.float32)
        for j in range(NJC):
            w = min(128, N - j * 128)
            sf[:w, j * OUT : (j + 1) * OUT] = S[j * 128 : j * 128 + w]
        return sf.astype(bf16)

    sfa = make_sf(S3)
    sfb = make_sf(S1)
    eye = np.eye(128, dtype=bf16)

    # w_k folded into the adjacency cast
    Ascl = (A * w2[:, None, None]).astype(bf16)  # [3, N, N]
    Atscl = (A_t * wb[:, None, None]).astype(bf16)  # [9, N, N]

    idxs = np.full((128, 8 * NSTRIPE), -1, dtype=np.int16)
    for st in range(NSTRIPE):
        for j in range(STRIPE):
            for rep in range(8):
                idxs[j % 16 + 16 * rep, st * 8 + j // 16] = STRIPE * st + j

    in_maps = []
    for p in range(NCORES):
        r0 = p * RS
        adj = np.concatenate(
            [Ascl[:, r0 : r0 + RS, :], Atscl[:, r0 : r0 + RS, :]], axis=0
        )
        adjg = np.zeros((KTOT, RS, NP), dtype=bf16)
        adjg[:, :, :N] = adj
        # dir1 stationaries: [125, st*32 + (0:16 S3 | 16:32 S1)]
        sst = np.zeros((STRIPE, NSTRIPE * 32), dtype=np.float32)
        for st in range(NSTRIPE):
            rows = slice(r0 + st * STRIPE, r0 + (st + 1) * STRIPE)
            sst[:, st * 32 : st * 32 + OUT] = S3[rows]
            sst[:, st * 32 + OUT : st * 32 + 32] = S1[rows]
        in_maps.append(
            {
                "adjg": adjg,
                "idxs": idxs,
                "sst": sst.astype(bf16),
                "sfa": sfa,
                "sfb": sfb,
                "idt": eye,
            }
        )
    return in_maps


def kernel(feature, A, A_t, weight_b2, weight_b, W3, b3, W1, b1, **kw):
    feature = np.asarray(feature, dtype=np.float32)
    A = np.asarray(A, dtype=np.float32)
    A_t = np.asarray(A_t, dtype=np.float32)
    w2 = np.asarray(weight_b2, dtype=np.float32).reshape(K_A)
    wb = np.asarray(weight_b, dtype=np.float32).reshape(K_AT)
    W3 = np.asarray(W3, dtype=np.float32)
    W1 = np.asarray(W1, dtype=np.float32)
    b3 = np.asarray(b3, dtype=np.float32)
    b1 = np.asarray(b1, dtype=np.float32)

    if "nc" not in _CACHE:
        _CACHE["nc"] = build()
    nc = _CACHE["nc"]

    in_maps = _make_inputs(feature, A, A_t, w2, wb, W3, W1)
    _CACHE["in_maps"] = in_maps

    res = run_bass_kernel_spmd(nc, in_maps, core_ids=list(range(NCORES)))

    col_a = np.zeros((OUT, N), dtype=np.float32)
    col_b = np.zeros((OUT, N), dtype=np.float32)
    row_a = np.empty((OUT, N), dtype=np.float32)
    row_b = np.empty((OUT, N), dtype=np.float32)
    for p in range(NCORES):
        r = res.results[p]
        col_a += r["o1"][0:16]
        col_b += r["o1"][32:48]
        row_a[:, p * RS : (p + 1) * RS] = r["o2a"]
        row_b[:, p * RS : (p + 1) * RS] = r["o2b"]

    U1 = (col_a + row_a).T + b3
    U2 = (col_b + row_b).T + b1
    return np.concatenate([U1, U2], axis=1).astype(np.float32)


# revision 45
# speedup vs baseline: 1.8008x; 1.0094x over previous
"""FAME-GCN Trainium2 kernel, v3.

Computes, for merged adjacency final_A = temp + temp^T, temp = sum_k w_k A_k:
    U1 = final_A @ (feature @ W3) + b3
    U2 = final_A2 @ (feature @ W1) + b1
    out = concat(U1, U2, axis=1)          # [5000, 32]

Distribution: node rows sharded 625/core across 8 NeuronCores; the [16, N]
column-direction partials (temp^T S) are summed across cores on the host,
the row-direction results (temp S)[own rows] concatenate.

Host prep: w_k is folded into a bf16 cast of each adjacency (w_k * A_k),
so the on-device merge is a pure chain of 2x-mode bf16 tensor_adds and the
dir1 stationaries are plain support matrices.

Per core, 5 stripes of 125 rows:
  - the 12 scaled relation stripes arrive as [125, 5000] bf16 transfers
    spread over six DMA paths (sync-HWDGE, scalar-HWDGE, 4 SWDGE queues),
  - dir1 (temp^T S, both groups): per-relation matmuls against a stacked
    [125, 32] stationary (S3-half for A-relations, S1-half for A_t),
    PSUM-accumulated over all 12 relations, one flush per column block,
  - dir2 (temp S): bf16 tensor_add merge per group on DVE, then PE
    transposes (8 column chunks packed per PSUM bank, bulk-copied to SBUF
    by the scalar engine) feeding an S-stationary accumulation chain.
"""

import sys

if "/opt/trn_rl_repo" not in sys.path:
    sys.path.insert(0, "/opt/trn_rl_repo")

import ml_dtypes
import numpy as np

import concourse.bacc as bacc
import concourse.mybir as mybir
from concourse.tile import TileContext
from concourse.bass_utils import run_bass_kernel_spmd

F32 = mybir.dt.float32
BF16 = mybir.dt.bfloat16

N = 5000
NP = 5120  # padded row length for the gather path (row bytes % 256 == 0)
OUT = 16
K_A, K_AT = 3, 9
KTOT = K_A + K_AT
NCORES = 8
RS = N // NCORES  # 625 rows per core
STRIPE = 125
NSTRIPE = RS // STRIPE  # 5
CB = 512
NCB = (N + CB - 1) // CB  # 10
NJC = (N + 127) // 128  # 40
JPACK = 8  # transposed 128-col chunks packed per PSUM bank
HALF = NP // 2  # 2560: gather half-width (= 5 CB blocks)

_CACHE = {}


def _c_blocks():
    return [(cb * CB, min(CB, N - cb * CB)) for cb in range(NCB)]


def _j_chunks():
    return [(j, min(128, N - j * 128)) for j in range(NJC)]


def build():
    nc = bacc.Bacc(num_swdge_queues=4)

    adjg = nc.declare_dram_parameter("adjg", [KTOT, RS, NP], BF16, isOutput=False)
    idxs = nc.declare_dram_parameter("idxs", [128, 8 * NSTRIPE], mybir.dt.int16, isOutput=False)
    sst = nc.declare_dram_parameter("sst", [STRIPE, NSTRIPE * 32], BF16, isOutput=False)
    sfa = nc.declare_dram_parameter("sfa", [128, NJC * OUT], BF16, isOutput=False)
    sfb = nc.declare_dram_parameter("sfb", [128, NJC * OUT], BF16, isOutput=False)
    idt = nc.declare_dram_parameter("idt", [128, 128], BF16, isOutput=False)

    o1 = nc.declare_dram_parameter("o1", [48, N], F32, isOutput=True)
    o2a = nc.declare_dram_parameter("o2a", [OUT, RS], F32, isOutput=True)
    o2b = nc.declare_dram_parameter("o2b", [OUT, RS], F32, isOutput=True)

    # group-a relations (k<3) via sync-HWDGE; group-b via SWDGE gathers
    # spread over the 4 SWDGE queues

    with TileContext(nc) as tc:
        with (
            tc.tile_pool(name="persist", bufs=1) as pp,
            tc.tile_pool(name="rawa", bufs=4) as rawap,
            tc.tile_pool(name="rawg", bufs=19) as rawgp,
            tc.tile_pool(name="mrg", bufs=1) as mrgp,
            tc.tile_pool(name="mrgb", bufs=2) as mrgbp,
            tc.tile_pool(name="strip", bufs=3) as stripp,
            tc.tile_pool(name="pdir", bufs=3, space="PSUM") as pdirp,
            tc.tile_pool(name="pt", bufs=2, space="PSUM") as ptp,
            tc.tile_pool(name="pd2", bufs=2, space="PSUM") as pd2p,
        ):
            # ---------------- persistent tiles ----------------
            sst_t = pp.tile([STRIPE, NSTRIPE * 32], BF16, tag="sst")
            nc.sync.dma_start(out=sst_t, in_=sst[:, :])
            sfa_t = pp.tile([128, NJC * OUT], BF16, tag="sfa")
            nc.sync.dma_start(out=sfa_t, in_=sfa[:, :])
            sfb_t = pp.tile([128, NJC * OUT], BF16, tag="sfb")
            nc.sync.dma_start(out=sfb_t, in_=sfb[:, :])
            id_t = pp.tile([128, 128], BF16, tag="idt")
            nc.sync.dma_start(out=id_t, in_=idt[:, :])
            ix = pp.tile([128, 8 * NSTRIPE], mybir.dt.int16, tag="ix")
            nc.sync.dma_start(out=ix, in_=idxs[:, :])

            o1sb = pp.tile([48, N], F32, tag="o1sb")
            acc2a = pp.tile([OUT, RS], F32, tag="acc2a")
            acc2b = pp.tile([OUT, RS], F32, tag="acc2b")

            for st in range(NSTRIPE):
                r0 = st * STRIPE
                # ---- loads: full-row gathers for group a, half-row gathers
                # (aligned to CB blocks 0-4 / 5-9) for group b ----
                raw = {}
                qn = st  # rotate queue assignment across stripes
                for k in range(K_A):
                    t = rawap.tile(
                        [128, 1, NP], BF16, tag="trawa", name=f"t_{st}_{k}"
                    )
                    nc.gpsimd.dma_gather(
                        t,
                        adjg[k, :, :],
                        ix[:, st * 8 : (st + 1) * 8],
                        128,
                        128,
                        NP,
                        elem_step=NP,
                        queue_num=qn % 4,
                    )
                    qn += 1
                    raw[k] = t
                for h in range(2):
                    for k in range(K_A, KTOT):
                        t = rawgp.tile(
                            [128, 1, HALF], BF16, tag="trawg", name=f"t_{st}_{k}_{h}"
                        )
                        nc.gpsimd.dma_gather(
                            t,
                            adjg[k, :, h * HALF : (h + 1) * HALF],
                            ix[:, st * 8 : (st + 1) * 8],
                            128,
                            128,
                            HALF,
                            elem_step=NP,
                            queue_num=qn % 4,
                        )
                        qn += 1
                        raw[(k, h)] = t

                def rawsl(k, c0, cw):
                    if k < K_A:
                        return raw[k][:STRIPE, 0, c0 : c0 + cw]
                    h = 0 if c0 < HALF else 1
                    assert c0 + cw <= HALF or c0 >= HALF
                    return raw[(k, h)][:STRIPE, 0, c0 - h * HALF : c0 - h * HALF + cw]

                # ---- dir1: two independent PSUM chains (group a from the
                # sync tile, group b from gathers) into disjoint partition
                # ranges of one bank; a completes early, freeing tsync ----
                sa = sst_t[:, st * 32 : st * 32 + OUT]
                sb = sst_t[:, st * 32 + OUT : st * 32 + 32]
                for cb, (c0, cw) in enumerate(_c_blocks()):
                    pd = pdirp.tile([48, CB], F32, tag="pd", name=f"pd_{st}_{cb}")
                    for k in range(K_A):
                        nc.tensor.matmul(
                            pd[0:OUT, :cw],
                            sa,
                            rawsl(k, c0, cw),
                            start=(k == 0),
                            stop=(k == K_A - 1),
                        )
                    for k in range(K_A, KTOT):
                        nc.tensor.matmul(
                            pd[32:48, :cw],
                            sb,
                            rawsl(k, c0, cw),
                            start=(k == K_A),
                            stop=(k == KTOT - 1),
                        )
                    dst = o1sb[:, c0 : c0 + cw]
                    if st == 0:
                        nc.vector.tensor_copy(out=dst, in_=pd[:, :cw])
                    else:
                        nc.vector.tensor_add(dst, dst, pd[:, :cw])

                # ---- merge per group: mrg = sum_k (w_k A_k) (bf16 adds) ----
                mrga = mrgp.tile([STRIPE, N], BF16, tag="mrga", name=f"mrga_{st}")
                mrgb = mrgbp.tile([STRIPE, N], BF16, tag="mrgb", name=f"mrgb_{st}")
                nc.vector.tensor_add(mrga, rawsl(0, 0, N), rawsl(1, 0, N))
                nc.vector.tensor_add(mrga, mrga, rawsl(2, 0, N))
                for h, c0, cw in ((0, 0, HALF), (1, HALF, N - HALF)):
                    dst = mrgb[:, c0 : c0 + cw]
                    nc.vector.tensor_add(
                        dst, rawsl(3, c0, cw), rawsl(4, c0, cw)
                    )
                    for k in range(5, KTOT):
                        nc.vector.tensor_add(dst, dst, rawsl(k, c0, cw))

                # ---- dir2 per group: acc2[:, st] = (mrg @ S)^T ----
                for gname, mrg, sf_t, acc2 in (
                    ("a", mrga, sfa_t, acc2a),
                    ("b", mrgb, sfb_t, acc2b),
                ):
                    pd2 = pd2p.tile(
                        [OUT, 126], F32, tag="pd2", name=f"pd2_{st}_{gname}"
                    )
                    jcs = _j_chunks()
                    for jb in range(0, NJC, JPACK):
                        chunk = jcs[jb : jb + JPACK]
                        pt = ptp.tile(
                            [128, JPACK * 126],
                            BF16,
                            tag="pt",
                            name=f"pt_{st}_{gname}_{jb}",
                        )
                        for jj, (j, cjw) in enumerate(chunk):
                            nc.tensor.transpose(
                                pt[:cjw, jj * 126 : jj * 126 + 126],
                                mrg[:STRIPE, 128 * j : 128 * j + cjw],
                                id_t[:STRIPE, :126],
                            )
                        strip = stripp.tile(
                            [128, JPACK * 126],
                            BF16,
                            tag="tt",
                            name=f"tt_{st}_{gname}_{jb}",
                        )
                        wid = len(chunk) * 126
                        nc.scalar.copy(out=strip[:, :wid], in_=pt[:, :wid])
                        for jj, (j, cjw) in enumerate(chunk):
                            nc.tensor.matmul(
                                pd2[:, :126],
                                sf_t[:cjw, j * OUT : (j + 1) * OUT],
                                strip[:cjw, jj * 126 : jj * 126 + 126],
                                start=(j == 0),
                                stop=(j == NJC - 1),
                            )
                    nc.vector.tensor_copy(
                        out=acc2[:, r0 : r0 + STRIPE], in_=pd2[:, :STRIPE]
                    )

            nc.sync.dma_start(out=o1[:, :], in_=o1sb)
            nc.sync.dma_start(out=o2a[:, :], in_=acc2a)
            nc.sync.dma_start(out=o2b[:, :], in_=acc2b)

    nc.compile()
    return nc


def _make_inputs(feature, A, A_t, w2, wb, W3, W1):
    bf16 = ml_dtypes.bfloat16

    S3 = (feature @ W3).astype(np.float32)  # [N, 16]
    S1 = (feature @ W1).astype(np.float32)

    # sf: S laid out [128, NJC*16]; sf[p, j*16+o] = S[j*128+p, o]
    def make_sf(S):
        sf = np.zeros((128, NJC * OUT), dtype=np.float32)
        for j in range(NJC):
            w = min(128, N - j * 128)
            sf[:w, j * OUT : (j + 1) * OUT] = S[j * 128 : j * 128 + w]
        return sf.astype(bf16)

    sfa = make_sf(S3)
    sfb = make_sf(S1)
    eye = np.eye(128, dtype=bf16)

    # w_k folded into the adjacency cast
    Ascl = (A * w2[:, None, None]).astype(bf16)  # [3, N, N]
    Atscl = (A_t * wb[:, None, None]).astype(bf16)  # [9, N, N]

    idxs = np.full((128, 8 * NSTRIPE), -1, dtype=np.int16)
    for st in range(NSTRIPE):
        for j in range(STRIPE):
            for rep in range(8):
                idxs[j % 16 + 16 * rep, st * 8 + j // 16] = STRIPE * st + j

    in_maps = []
    for p in range(NCORES):
        r0 = p * RS
        adj = np.concatenate(
            [Ascl[:, r0 : r0 + RS, :], Atscl[:, r0 : r0 + RS, :]], axis=0
        )
        adjg = np.zeros((KTOT, RS, NP), dtype=bf16)
        adjg[:, :, :N] = adj
        # dir1 stationaries: [125, st*32 + (0:16 S3 | 16:32 S1)]
        sst = np.zeros((STRIPE, NSTRIPE * 32), dtype=np.float32)
        for st in range(NSTRIPE):
            rows = slice(r0 + st * STRIPE, r0 + (st + 1) * STRIPE)
            sst[:, st * 32 : st * 32 + OUT] = S3[rows]
            sst[:, st * 32 + OUT : st * 32 + 32] = S1[rows]
        in_maps.append(
            {
                "adjg": adjg,
                "idxs": idxs,
                "sst": sst.astype(bf16),
                "sfa": sfa,
                "sfb": sfb,
                "idt": eye,
            }
        )
    return in_maps


def kernel(feature, A, A_t, weight_b2, weight_b, W3, b3, W1, b1, **kw):
    feature = np.asarray(feature, dtype=np.float32)
    A = np.asarray(A, dtype=np.float32)
    A_t = np.asarray(A_t, dtype=np.float32)
    w2 = np.asarray(weight_b2, dtype=np.float32).reshape(K_A)
    wb = np.asarray(weight_b, dtype=np.float32).reshape(K_AT)
    W3 = np.asarray(W3, dtype=np.float32)
    W1 = np.asarray(W1, dtype=np.float32)
    b3 = np.asarray(b3, dtype=np.float32)
    b1 = np.asarray(b1, dtype=np.float32)

    if "nc" not in _CACHE:
        _CACHE["nc"] = build()
    nc = _CACHE["nc"]

    in_maps = _make_inputs(feature, A, A_t, w2, wb, W3, W1)
    _CACHE["in_maps"] = in_maps

    res = run_bass_kernel_spmd(nc, in_maps, core_ids=list(range(NCORES)))

    col_a = np.zeros((OUT, N), dtype=np.float32)
    col_b = np.zeros((OUT, N), dtype=np.float32)
    row_a = np.empty((OUT, N), dtype=np.float32)
    row_b = np.empty((OUT, N), dtype=np.float32)
    for p in range(NCORES):
        r = res.results[p]
        col_a += r["o1"][0:16]
        col_b += r["o1"][32:48]
        row_a[:, p * RS : (p + 1) * RS] = r["o2a"]
        row_b[:, p * RS : (p + 1) * RS] = r["o2b"]

    U1 = (col_a + row_a).T + b3
    U2 = (col_b + row_b).T + b1
    return np.concatenate([U1, U2], axis=1).astype(np.float32)


# revision 51
# speedup vs baseline: 2.0616x; 1.1448x over previous
"""FAME-GCN Trainium2 kernel, v3.

Computes, for merged adjacency final_A = temp + temp^T, temp = sum_k w_k A_k:
    U1 = final_A @ (feature @ W3) + b3
    U2 = final_A2 @ (feature @ W1) + b1
    out = concat(U1, U2, axis=1)          # [5000, 32]

Distribution: node rows sharded 625/core across 8 NeuronCores; the [16, N]
column-direction partials (temp^T S) are summed across cores on the host,
the row-direction results (temp S)[own rows] concatenate.

Host prep: w_k is folded into a bf16 cast of each adjacency (w_k * A_k),
so the on-device merge is a pure chain of 2x-mode bf16 tensor_adds and the
dir1 stationaries are plain support matrices.

Per core, 5 stripes of 125 rows:
  - the 12 scaled relation stripes arrive as [125, 5000] bf16 transfers
    spread over six DMA paths (sync-HWDGE, scalar-HWDGE, 4 SWDGE queues),
  - dir1 (temp^T S, both groups): per-relation matmuls against a stacked
    [125, 32] stationary (S3-half for A-relations, S1-half for A_t),
    PSUM-accumulated over all 12 relations, one flush per column block,
  - dir2 (temp S): bf16 tensor_add merge per group on DVE, then PE
    transposes (8 column chunks packed per PSUM bank, bulk-copied to SBUF
    by the scalar engine) feeding an S-stationary accumulation chain.
"""

import sys

if "/opt/trn_rl_repo" not in sys.path:
    sys.path.insert(0, "/opt/trn_rl_repo")

import ml_dtypes
import numpy as np

import concourse.bacc as bacc
import concourse.mybir as mybir
from concourse.tile import TileContext
from concourse.bass_utils import run_bass_kernel_spmd

F32 = mybir.dt.float32
BF16 = mybir.dt.bfloat16

N = 5000
NP = 5120  # padded row length for the gather path (row bytes % 256 == 0)
OUT = 16
K_A, K_AT = 3, 9
KTOT = K_A + K_AT
NCORES = 8
RS = N // NCORES  # 625 rows per core
STRIPE = 125
NSTRIPE = RS // STRIPE  # 5
CB = 512
NCB = (N + CB - 1) // CB  # 10
NJC = (N + 127) // 128  # 40
JPACK = 8  # transposed 128-col chunks packed per PSUM bank
HALF = NP // 2  # 2560: gather half-width (= 5 CB blocks)

_CACHE = {}


def _c_blocks():
    return [(cb * CB, min(CB, N - cb * CB)) for cb in range(NCB)]


def _j_chunks():
    return [(j, min(128, N - j * 128)) for j in range(NJC)]


def build():
    nc = bacc.Bacc(num_swdge_queues=4)

    adjg = nc.declare_dram_parameter("adjg", [KTOT, RS, NP], BF16, isOutput=False)
    idxs = nc.declare_dram_parameter("idxs", [128, 8 * NSTRIPE], mybir.dt.int16, isOutput=False)
    sst = nc.declare_dram_parameter("sst", [STRIPE, NSTRIPE * 32], BF16, isOutput=False)
    sfa = nc.declare_dram_parameter("sfa", [128, NJC * OUT], BF16, isOutput=False)
    sfb = nc.declare_dram_parameter("sfb", [128, NJC * OUT], BF16, isOutput=False)
    idt = nc.declare_dram_parameter("idt", [128, 128], BF16, isOutput=False)

    o1 = nc.declare_dram_parameter("o1", [48, N], BF16, isOutput=True)
    o2a = nc.declare_dram_parameter("o2a", [OUT, RS], F32, isOutput=True)
    o2b = nc.declare_dram_parameter("o2b", [OUT, RS], F32, isOutput=True)

    # group-a relations (k<3) via sync-HWDGE; group-b via SWDGE gathers
    # spread over the 4 SWDGE queues

    with TileContext(nc) as tc:
        with (
            tc.tile_pool(name="persist", bufs=1) as pp,
            tc.tile_pool(name="rawa", bufs=3) as rawap,
            tc.tile_pool(name="rawg", bufs=20) as rawgp,
            tc.tile_pool(name="mrg", bufs=1) as mrgp,
            tc.tile_pool(name="mrgb", bufs=2) as mrgbp,
            tc.tile_pool(name="strip", bufs=3) as stripp,
            tc.tile_pool(name="pdir", bufs=3, space="PSUM") as pdirp,
            tc.tile_pool(name="pt", bufs=3, space="PSUM") as ptp,
            tc.tile_pool(name="pd2", bufs=2, space="PSUM") as pd2p,
        ):
            # ---------------- persistent tiles ----------------
            sst_t = pp.tile([STRIPE, NSTRIPE * 32], BF16, tag="sst")
            nc.sync.dma_start(out=sst_t, in_=sst[:, :])
            sfa_t = pp.tile([128, NJC * OUT], BF16, tag="sfa")
            nc.sync.dma_start(out=sfa_t, in_=sfa[:, :])
            sfb_t = pp.tile([128, NJC * OUT], BF16, tag="sfb")
            nc.sync.dma_start(out=sfb_t, in_=sfb[:, :])
            id_t = pp.tile([128, 128], BF16, tag="idt")
            nc.sync.dma_start(out=id_t, in_=idt[:, :])
            ix = pp.tile([128, 8 * NSTRIPE], mybir.dt.int16, tag="ix")
            nc.sync.dma_start(out=ix, in_=idxs[:, :])

            o1sb = pp.tile([48, N], BF16, tag="o1sb")
            acc2a = pp.tile([OUT, RS], F32, tag="acc2a")
            acc2b = pp.tile([OUT, RS], F32, tag="acc2b")

            for st in range(NSTRIPE):
                r0 = st * STRIPE
                # ---- loads: full-row gathers for group a, half-row gathers
                # (aligned to CB blocks 0-4 / 5-9) for group b ----
                raw = {}
                qn = st  # rotate queue assignment across stripes
                for k in range(K_A):
                    t = rawap.tile(
                        [128, 1, NP], BF16, tag="trawa", name=f"t_{st}_{k}"
                    )
                    nc.gpsimd.dma_gather(
                        t,
                        adjg[k, :, :],
                        ix[:, st * 8 : (st + 1) * 8],
                        128,
                        128,
                        NP,
                        elem_step=NP,
                        queue_num=qn % 4,
                    )
                    qn += 1
                    raw[k] = t
                for h in range(2):
                    for k in range(K_A, KTOT):
                        t = rawgp.tile(
                            [128, 1, HALF], BF16, tag="trawg", name=f"t_{st}_{k}_{h}"
                        )
                        nc.gpsimd.dma_gather(
                            t,
                            adjg[k, :, h * HALF : (h + 1) * HALF],
                            ix[:, st * 8 : (st + 1) * 8],
                            128,
                            128,
                            HALF,
                            elem_step=NP,
                            queue_num=qn % 4,
                        )
                        qn += 1
                        raw[(k, h)] = t

                def rawsl(k, c0, cw):
                    if k < K_A:
                        return raw[k][:STRIPE, 0, c0 : c0 + cw]
                    h = 0 if c0 < HALF else 1
                    assert c0 + cw <= HALF or c0 >= HALF
                    return raw[(k, h)][:STRIPE, 0, c0 - h * HALF : c0 - h * HALF + cw]

                # ---- dir1: two independent PSUM chains (group a from the
                # sync tile, group b from gathers) into disjoint partition
                # ranges of one bank; a completes early, freeing tsync ----
                sa = sst_t[:, st * 32 : st * 32 + OUT]
                sb = sst_t[:, st * 32 + OUT : st * 32 + 32]
                for cb, (c0, cw) in enumerate(_c_blocks()):
                    pd = pdirp.tile([48, CB], F32, tag="pd", name=f"pd_{st}_{cb}")
                    for k in range(K_A):
                        nc.tensor.matmul(
                            pd[0:OUT, :cw],
                            sa,
                            rawsl(k, c0, cw),
                            start=(k == 0),
                            stop=(k == K_A - 1),
                        )
                    for k in range(K_A, KTOT):
                        nc.tensor.matmul(
                            pd[32:48, :cw],
                            sb,
                            rawsl(k, c0, cw),
                            start=(k == K_A),
                            stop=(k == KTOT - 1),
                        )
                    dst = o1sb[:, c0 : c0 + cw]
                    if st == 0:
                        nc.vector.tensor_copy(out=dst, in_=pd[:, :cw])
                    else:
                        nc.vector.tensor_add(dst, dst, pd[:, :cw])

                # ---- merge per group: mrg = sum_k (w_k A_k) (bf16 adds) ----
                mrga = mrgp.tile([STRIPE, N], BF16, tag="mrga", name=f"mrga_{st}")
                mrgb = mrgbp.tile([STRIPE, N], BF16, tag="mrgb", name=f"mrgb_{st}")
                nc.vector.tensor_add(mrga, rawsl(0, 0, N), rawsl(1, 0, N))
                nc.vector.tensor_add(mrga, mrga, rawsl(2, 0, N))
                for h, c0, cw in ((0, 0, HALF), (1, HALF, N - HALF)):
                    dst = mrgb[:, c0 : c0 + cw]
                    nc.vector.tensor_add(
                        dst, rawsl(3, c0, cw), rawsl(4, c0, cw)
                    )
                    for k in range(5, KTOT):
                        nc.vector.tensor_add(dst, dst, rawsl(k, c0, cw))

                # ---- dir2 per group: acc2[:, st] = (mrg @ S)^T ----
                for gname, mrg, sf_t, acc2 in (
                    ("a", mrga, sfa_t, acc2a),
                    ("b", mrgb, sfb_t, acc2b),
                ):
                    pd2 = pd2p.tile(
                        [OUT, 126], F32, tag="pd2", name=f"pd2_{st}_{gname}"
                    )
                    jcs = _j_chunks()
                    for jb in range(0, NJC, JPACK):
                        chunk = jcs[jb : jb + JPACK]
                        pt = ptp.tile(
                            [128, JPACK * 126],
                            BF16,
                            tag="pt",
                            name=f"pt_{st}_{gname}_{jb}",
                        )
                        for jj, (j, cjw) in enumerate(chunk):
                            nc.tensor.transpose(
                                pt[:cjw, jj * 126 : jj * 126 + 126],
                                mrg[:STRIPE, 128 * j : 128 * j + cjw],
                                id_t[:STRIPE, :126],
                            )
                        strip = stripp.tile(
                            [128, JPACK * 126],
                            BF16,
                            tag="tt",
                            name=f"tt_{st}_{gname}_{jb}",
                        )
                        wid = len(chunk) * 126
                        nc.scalar.copy(out=strip[:, :wid], in_=pt[:, :wid])
                        for jj, (j, cjw) in enumerate(chunk):
                            nc.tensor.matmul(
                                pd2[:, :126],
                                sf_t[:cjw, j * OUT : (j + 1) * OUT],
                                strip[:cjw, jj * 126 : jj * 126 + 126],
                                start=(j == 0),
                                stop=(j == NJC - 1),
                            )
                    nc.vector.tensor_copy(
                        out=acc2[:, r0 : r0 + STRIPE], in_=pd2[:, :STRIPE]
                    )
                    o2 = o2a if gname == "a" else o2b
                    nc.sync.dma_start(
                        out=o2[:, r0 : r0 + STRIPE],
                        in_=acc2[:, r0 : r0 + STRIPE],
                    )

            nc.sync.dma_start(out=o1[:, :], in_=o1sb)

    nc.compile()
    return nc


def _make_inputs(feature, A, A_t, w2, wb, W3, W1):
    bf16 = ml_dtypes.bfloat16

    S3 = (feature @ W3).astype(np.float32)  # [N, 16]
    S1 = (feature @ W1).astype(np.float32)

    # sf: S laid out [128, NJC*16]; sf[p, j*16+o] = S[j*128+p, o]
    def make_sf(S):
        sf = np.zeros((128, NJC * OUT), dtype=np.float32)
        for j in range(NJC):
            w = min(128, N - j * 128)
            sf[:w, j * OUT : (j + 1) * OUT] = S[j * 128 : j * 128 + w]
        return sf.astype(bf16)

    sfa = make_sf(S3)
    sfb = make_sf(S1)
    eye = np.eye(128, dtype=bf16)

    # w_k folded into the adjacency cast
    Ascl = (A * w2[:, None, None]).astype(bf16)  # [3, N, N]
    Atscl = (A_t * wb[:, None, None]).astype(bf16)  # [9, N, N]

    idxs = np.full((128, 8 * NSTRIPE), -1, dtype=np.int16)
    for st in range(NSTRIPE):
        for j in range(STRIPE):
            for rep in range(8):
                idxs[j % 16 + 16 * rep, st * 8 + j // 16] = STRIPE * st + j

    in_maps = []
    for p in range(NCORES):
        r0 = p * RS
        adj = np.concatenate(
            [Ascl[:, r0 : r0 + RS, :], Atscl[:, r0 : r0 + RS, :]], axis=0
        )
        adjg = np.zeros((KTOT, RS, NP), dtype=bf16)
        adjg[:, :, :N] = adj
        # dir1 stationaries: [125, st*32 + (0:16 S3 | 16:32 S1)]
        sst = np.zeros((STRIPE, NSTRIPE * 32), dtype=np.float32)
        for st in range(NSTRIPE):
            rows = slice(r0 + st * STRIPE, r0 + (st + 1) * STRIPE)
            sst[:, st * 32 : st * 32 + OUT] = S3[rows]
            sst[:, st * 32 + OUT : st * 32 + 32] = S1[rows]
        in_maps.append(
            {
                "adjg": adjg,
                "idxs": idxs,
                "sst": sst.astype(bf16),
                "sfa": sfa,
                "sfb": sfb,
                "idt": eye,
            }
        )
    return in_maps


def kernel(feature, A, A_t, weight_b2, weight_b, W3, b3, W1, b1, **kw):
    feature = np.asarray(feature, dtype=np.float32)
    A = np.asarray(A, dtype=np.float32)
    A_t = np.asarray(A_t, dtype=np.float32)
    w2 = np.asarray(weight_b2, dtype=np.float32).reshape(K_A)
    wb = np.asarray(weight_b, dtype=np.float32).reshape(K_AT)
    W3 = np.asarray(W3, dtype=np.float32)
    W1 = np.asarray(W1, dtype=np.float32)
    b3 = np.asarray(b3, dtype=np.float32)
    b1 = np.asarray(b1, dtype=np.float32)

    if "nc" not in _CACHE:
        _CACHE["nc"] = build()
    nc = _CACHE["nc"]

    in_maps = _make_inputs(feature, A, A_t, w2, wb, W3, W1)
    _CACHE["in_maps"] = in_maps

    res = run_bass_kernel_spmd(nc, in_maps, core_ids=list(range(NCORES)))

    col_a = np.zeros((OUT, N), dtype=np.float32)
    col_b = np.zeros((OUT, N), dtype=np.float32)
    row_a = np.empty((OUT, N), dtype=np.float32)
    row_b = np.empty((OUT, N), dtype=np.float32)
    for p in range(NCORES):
        r = res.results[p]
        col_a += r["o1"][0:16].astype(np.float32)
        col_b += r["o1"][32:48].astype(np.float32)
        row_a[:, p * RS : (p + 1) * RS] = r["o2a"]
        row_b[:, p * RS : (p + 1) * RS] = r["o2b"]

    U1 = (col_a + row_a).T + b3
    U2 = (col_b + row_b).T + b1
    return np.concatenate([U1, U2], axis=1).astype(np.float32)


# revision 52
# speedup vs baseline: 2.1713x; 1.0532x over previous
"""FAME-GCN Trainium2 kernel, v3.

Computes, for merged adjacency final_A = temp + temp^T, temp = sum_k w_k A_k:
    U1 = final_A @ (feature @ W3) + b3
    U2 = final_A2 @ (feature @ W1) + b1
    out = concat(U1, U2, axis=1)          # [5000, 32]

Distribution: node rows sharded 625/core across 8 NeuronCores; the [16, N]
column-direction partials (temp^T S) are summed across cores on the host,
the row-direction results (temp S)[own rows] concatenate.

Host prep: w_k is folded into a bf16 cast of each adjacency (w_k * A_k),
so the on-device merge is a pure chain of 2x-mode bf16 tensor_adds and the
dir1 stationaries are plain support matrices.

Per core, 5 stripes of 125 rows:
  - the 12 scaled relation stripes arrive as [125, 5000] bf16 transfers
    spread over six DMA paths (sync-HWDGE, scalar-HWDGE, 4 SWDGE queues),
  - dir1 (temp^T S, both groups): per-relation matmuls against a stacked
    [125, 32] stationary (S3-half for A-relations, S1-half for A_t),
    PSUM-accumulated over all 12 relations, one flush per column block,
  - dir2 (temp S): bf16 tensor_add merge per group on DVE, then PE
    transposes (8 column chunks packed per PSUM bank, bulk-copied to SBUF
    by the scalar engine) feeding an S-stationary accumulation chain.
"""

import sys

if "/opt/trn_rl_repo" not in sys.path:
    sys.path.insert(0, "/opt/trn_rl_repo")

import ml_dtypes
import numpy as np

import concourse.bacc as bacc
import concourse.mybir as mybir
from concourse.tile import TileContext
from concourse.bass_utils import run_bass_kernel_spmd

F32 = mybir.dt.float32
BF16 = mybir.dt.bfloat16

N = 5000
NP = 5120  # padded row length for the gather path (row bytes % 256 == 0)
OUT = 16
K_A, K_AT = 3, 9
KTOT = K_A + K_AT
NCORES = 8
RS = N // NCORES  # 625 rows per core
STRIPE = 125
NSTRIPE = RS // STRIPE  # 5
CB = 512
NCB = (N + CB - 1) // CB  # 10
NJC = (N + 127) // 128  # 40
JPACK = 8  # transposed 128-col chunks packed per PSUM bank
HALF = NP // 2  # 2560: gather half-width (= 5 CB blocks)

_CACHE = {}


def _c_blocks():
    return [(cb * CB, min(CB, N - cb * CB)) for cb in range(NCB)]


def _j_chunks():
    return [(j, min(128, N - j * 128)) for j in range(NJC)]


def build():
    nc = bacc.Bacc(num_swdge_queues=4)

    adjg = nc.declare_dram_parameter("adjg", [KTOT, RS, NP], BF16, isOutput=False)
    idxs = nc.declare_dram_parameter("idxs", [128, 8 * NSTRIPE], mybir.dt.int16, isOutput=False)
    sst = nc.declare_dram_parameter("sst", [STRIPE, NSTRIPE * 32], BF16, isOutput=False)
    sfa = nc.declare_dram_parameter("sfa", [128, NJC * OUT], BF16, isOutput=False)
    sfb = nc.declare_dram_parameter("sfb", [128, NJC * OUT], BF16, isOutput=False)
    idt = nc.declare_dram_parameter("idt", [128, 128], BF16, isOutput=False)

    o1 = nc.declare_dram_parameter("o1", [48, N], BF16, isOutput=True)
    o2a = nc.declare_dram_parameter("o2a", [OUT, RS], F32, isOutput=True)
    o2b = nc.declare_dram_parameter("o2b", [OUT, RS], F32, isOutput=True)

    # group-a relations (k<3) via sync-HWDGE; group-b via SWDGE gathers
    # spread over the 4 SWDGE queues

    with TileContext(nc) as tc:
        with (
            tc.tile_pool(name="persist", bufs=1) as pp,
            tc.tile_pool(name="rawa", bufs=4) as rawap,
            tc.tile_pool(name="rawg", bufs=20) as rawgp,
            tc.tile_pool(name="mrg", bufs=1) as mrgp,
            tc.tile_pool(name="mrgb", bufs=2) as mrgbp,
            tc.tile_pool(name="strip", bufs=3) as stripp,
            tc.tile_pool(name="pdir", bufs=3, space="PSUM") as pdirp,
            tc.tile_pool(name="pt", bufs=3, space="PSUM") as ptp,
            tc.tile_pool(name="pd2", bufs=2, space="PSUM") as pd2p,
        ):
            # ---------------- persistent tiles ----------------
            sst_t = pp.tile([STRIPE, NSTRIPE * 32], BF16, tag="sst")
            nc.sync.dma_start(out=sst_t, in_=sst[:, :])
            sfa_t = pp.tile([128, NJC * OUT], BF16, tag="sfa")
            nc.sync.dma_start(out=sfa_t, in_=sfa[:, :])
            sfb_t = pp.tile([128, NJC * OUT], BF16, tag="sfb")
            nc.sync.dma_start(out=sfb_t, in_=sfb[:, :])
            id_t = pp.tile([128, 128], BF16, tag="idt")
            nc.sync.dma_start(out=id_t, in_=idt[:, :])
            ix = pp.tile([128, 8 * NSTRIPE], mybir.dt.int16, tag="ix")
            nc.sync.dma_start(out=ix, in_=idxs[:, :])

            o1sb = pp.tile([48, N], BF16, tag="o1sb")
            acc2a = pp.tile([OUT, RS], F32, tag="acc2a")
            acc2b = pp.tile([OUT, RS], F32, tag="acc2b")

            for st in range(NSTRIPE):
                r0 = st * STRIPE
                # ---- loads: full-row gathers for group a, half-row gathers
                # (aligned to CB blocks 0-4 / 5-9) for group b ----
                raw = {}
                qn = st  # rotate queue assignment across stripes
                for k in range(K_A):
                    t = rawap.tile(
                        [128, 1, NP], BF16, tag="trawa", name=f"t_{st}_{k}"
                    )
                    nc.gpsimd.dma_gather(
                        t,
                        adjg[k, :, :],
                        ix[:, st * 8 : (st + 1) * 8],
                        128,
                        128,
                        NP,
                        elem_step=NP,
                        queue_num=qn % 4,
                    )
                    qn += 1
                    raw[k] = t
                for h in range(2):
                    for k in range(K_A, KTOT):
                        t = rawgp.tile(
                            [128, 1, HALF], BF16, tag="trawg", name=f"t_{st}_{k}_{h}"
                        )
                        nc.gpsimd.dma_gather(
                            t,
                            adjg[k, :, h * HALF : (h + 1) * HALF],
                            ix[:, st * 8 : (st + 1) * 8],
                            128,
                            128,
                            HALF,
                            elem_step=NP,
                            queue_num=qn % 4,
                        )
                        qn += 1
                        raw[(k, h)] = t

                def rawsl(k, c0, cw):
                    if k < K_A:
                        return raw[k][:STRIPE, 0, c0 : c0 + cw]
                    h = 0 if c0 < HALF else 1
                    assert c0 + cw <= HALF or c0 >= HALF
                    return raw[(k, h)][:STRIPE, 0, c0 - h * HALF : c0 - h * HALF + cw]

                # ---- dir1: two independent PSUM chains (group a from the
                # sync tile, group b from gathers) into disjoint partition
                # ranges of one bank; a completes early, freeing tsync ----
                sa = sst_t[:, st * 32 : st * 32 + OUT]
                sb = sst_t[:, st * 32 + OUT : st * 32 + 32]
                for cb, (c0, cw) in enumerate(_c_blocks()):
                    pd = pdirp.tile([48, CB], F32, tag="pd", name=f"pd_{st}_{cb}")
                    for k in range(K_A):
                        nc.tensor.matmul(
                            pd[0:OUT, :cw],
                            sa,
                            rawsl(k, c0, cw),
                            start=(k == 0),
                            stop=(k == K_A - 1),
                        )
                    for k in range(K_A, KTOT):
                        nc.tensor.matmul(
                            pd[32:48, :cw],
                            sb,
                            rawsl(k, c0, cw),
                            start=(k == K_A),
                            stop=(k == KTOT - 1),
                        )
                    dst = o1sb[:, c0 : c0 + cw]
                    if st == 0:
                        nc.vector.tensor_copy(out=dst, in_=pd[:, :cw])
                    else:
                        nc.vector.tensor_add(dst, dst, pd[:, :cw])

                # ---- merge per group: mrg = sum_k (w_k A_k) (bf16 adds) ----
                mrga = mrgp.tile([STRIPE, N], BF16, tag="mrga", name=f"mrga_{st}")
                mrgb = mrgbp.tile([STRIPE, N], BF16, tag="mrgb", name=f"mrgb_{st}")
                nc.vector.tensor_add(mrga, rawsl(0, 0, N), rawsl(1, 0, N))
                nc.vector.tensor_add(mrga, mrga, rawsl(2, 0, N))
                for h, c0, cw in ((0, 0, HALF), (1, HALF, N - HALF)):
                    dst = mrgb[:, c0 : c0 + cw]
                    nc.vector.tensor_add(
                        dst, rawsl(3, c0, cw), rawsl(4, c0, cw)
                    )
                    for k in range(5, KTOT):
                        nc.vector.tensor_add(dst, dst, rawsl(k, c0, cw))

                # ---- dir2 per group: acc2[:, st] = (mrg @ S)^T ----
                for gname, mrg, sf_t, acc2 in (
                    ("a", mrga, sfa_t, acc2a),
                    ("b", mrgb, sfb_t, acc2b),
                ):
                    pd2 = pd2p.tile(
                        [OUT, 126], F32, tag="pd2", name=f"pd2_{st}_{gname}"
                    )
                    jcs = _j_chunks()
                    for jb in range(0, NJC, JPACK):
                        chunk = jcs[jb : jb + JPACK]
                        pt = ptp.tile(
                            [128, JPACK * 126],
                            BF16,
                            tag="pt",
                            name=f"pt_{st}_{gname}_{jb}",
                        )
                        for jj, (j, cjw) in enumerate(chunk):
                            nc.tensor.transpose(
                                pt[:cjw, jj * 126 : jj * 126 + 126],
                                mrg[:STRIPE, 128 * j : 128 * j + cjw],
                                id_t[:STRIPE, :126],
                            )
                        strip = stripp.tile(
                            [128, JPACK * 126],
                            BF16,
                            tag="tt",
                            name=f"tt_{st}_{gname}_{jb}",
                        )
                        wid = len(chunk) * 126
                        nc.scalar.copy(out=strip[:, :wid], in_=pt[:, :wid])
                        for jj, (j, cjw) in enumerate(chunk):
                            nc.tensor.matmul(
                                pd2[:, :126],
                                sf_t[:cjw, j * OUT : (j + 1) * OUT],
                                strip[:cjw, jj * 126 : jj * 126 + 126],
                                start=(j == 0),
                                stop=(j == NJC - 1),
                            )
                    nc.vector.tensor_copy(
                        out=acc2[:, r0 : r0 + STRIPE], in_=pd2[:, :STRIPE]
                    )
                    o2 = o2a if gname == "a" else o2b
                    nc.sync.dma_start(
                        out=o2[:, r0 : r0 + STRIPE],
                        in_=acc2[:, r0 : r0 + STRIPE],
                    )

            nc.sync.dma_start(out=o1[:, :], in_=o1sb)

    nc.compile()
    return nc


def _make_inputs(feature, A, A_t, w2, wb, W3, W1):
    bf16 = ml_dtypes.bfloat16

    S3 = (feature @ W3).astype(np.float32)  # [N, 16]
    S1 = (feature @ W1).astype(np.float32)

    # sf: S laid out [128, NJC*16]; sf[p, j*16+o] = S[j*128+p, o]
    def make_sf(S):
        sf = np.zeros((128, NJC * OUT), dtype=np.float32)
        for j in range(NJC):
            w = min(128, N - j * 128)
            sf[:w, j * OUT : (j + 1) * OUT] = S[j * 128 : j * 128 + w]
        return sf.astype(bf16)

    sfa = make_sf(S3)
    sfb = make_sf(S1)
    eye = np.eye(128, dtype=bf16)

    # w_k folded into the adjacency cast
    Ascl = (A * w2[:, None, None]).astype(bf16)  # [3, N, N]
    Atscl = (A_t * wb[:, None, None]).astype(bf16)  # [9, N, N]

    idxs = np.full((128, 8 * NSTRIPE), -1, dtype=np.int16)
    for st in range(NSTRIPE):
        for j in range(STRIPE):
            for rep in range(8):
                idxs[j % 16 + 16 * rep, st * 8 + j // 16] = STRIPE * st + j

    in_maps = []
    for p in range(NCORES):
        r0 = p * RS
        adj = np.concatenate(
            [Ascl[:, r0 : r0 + RS, :], Atscl[:, r0 : r0 + RS, :]], axis=0
        )
        adjg = np.zeros((KTOT, RS, NP), dtype=bf16)
        adjg[:, :, :N] = adj
        # dir1 stationaries: [125, st*32 + (0:16 S3 | 16:32 S1)]
        sst = np.zeros((STRIPE, NSTRIPE * 32), dtype=np.float32)
        for st in range(NSTRIPE):
            rows = slice(r0 + st * STRIPE, r0 + (st + 1) * STRIPE)
            sst[:, st * 32 : st * 32 + OUT] = S3[rows]
            sst[:, st * 32 + OUT : st * 32 + 32] = S1[rows]
        in_maps.append(
            {
                "adjg": adjg,
                "idxs": idxs,
                "sst": sst.astype(bf16),
                "sfa": sfa,
                "sfb": sfb,
                "idt": eye,
            }
        )
    return in_maps


def kernel(feature, A, A_t, weight_b2, weight_b, W3, b3, W1, b1, **kw):
    feature = np.asarray(feature, dtype=np.float32)
    A = np.asarray(A, dtype=np.float32)
    A_t = np.asarray(A_t, dtype=np.float32)
    w2 = np.asarray(weight_b2, dtype=np.float32).reshape(K_A)
    wb = np.asarray(weight_b, dtype=np.float32).reshape(K_AT)
    W3 = np.asarray(W3, dtype=np.float32)
    W1 = np.asarray(W1, dtype=np.float32)
    b3 = np.asarray(b3, dtype=np.float32)
    b1 = np.asarray(b1, dtype=np.float32)

    if "nc" not in _CACHE:
        _CACHE["nc"] = build()
    nc = _CACHE["nc"]

    in_maps = _make_inputs(feature, A, A_t, w2, wb, W3, W1)
    _CACHE["in_maps"] = in_maps

    res = run_bass_kernel_spmd(nc, in_maps, core_ids=list(range(NCORES)))

    col_a = np.zeros((OUT, N), dtype=np.float32)
    col_b = np.zeros((OUT, N), dtype=np.float32)
    row_a = np.empty((OUT, N), dtype=np.float32)
    row_b = np.empty((OUT, N), dtype=np.float32)
    for p in range(NCORES):
        r = res.results[p]
        col_a += r["o1"][0:16].astype(np.float32)
        col_b += r["o1"][32:48].astype(np.float32)
        row_a[:, p * RS : (p + 1) * RS] = r["o2a"]
        row_b[:, p * RS : (p + 1) * RS] = r["o2b"]

    U1 = (col_a + row_a).T + b3
    U2 = (col_b + row_b).T + b1
    return np.concatenate([U1, U2], axis=1).astype(np.float32)


# revision 54
# speedup vs baseline: 2.4161x; 1.1127x over previous
"""FAME-GCN Trainium2 kernel, v3.

Computes, for merged adjacency final_A = temp + temp^T, temp = sum_k w_k A_k:
    U1 = final_A @ (feature @ W3) + b3
    U2 = final_A2 @ (feature @ W1) + b1
    out = concat(U1, U2, axis=1)          # [5000, 32]

Distribution: node rows sharded 625/core across 8 NeuronCores; the [16, N]
column-direction partials (temp^T S) are summed across cores on the host,
the row-direction results (temp S)[own rows] concatenate.

Host prep: w_k is folded into a bf16 cast of each adjacency (w_k * A_k),
so the on-device merge is a pure chain of 2x-mode bf16 tensor_adds and the
dir1 stationaries are plain support matrices.

Per core, 5 stripes of 125 rows:
  - the 12 scaled relation stripes arrive as [125, 5000] bf16 transfers
    spread over six DMA paths (sync-HWDGE, scalar-HWDGE, 4 SWDGE queues),
  - dir1 (temp^T S, both groups): per-relation matmuls against a stacked
    [125, 32] stationary (S3-half for A-relations, S1-half for A_t),
    PSUM-accumulated over all 12 relations, one flush per column block,
  - dir2 (temp S): bf16 tensor_add merge per group on DVE, then PE
    transposes (8 column chunks packed per PSUM bank, bulk-copied to SBUF
    by the scalar engine) feeding an S-stationary accumulation chain.
"""

import sys

if "/opt/trn_rl_repo" not in sys.path:
    sys.path.insert(0, "/opt/trn_rl_repo")

import ml_dtypes
import numpy as np

import concourse.bacc as bacc
import concourse.mybir as mybir
from concourse.tile import TileContext
from concourse.bass_utils import run_bass_kernel_spmd

F32 = mybir.dt.float32
BF16 = mybir.dt.bfloat16

N = 5000
NP = 5120  # padded row length for the gather path (row bytes % 256 == 0)
OUT = 16
K_A, K_AT = 3, 9
KTOT = K_A + K_AT
NCORES = 8
RS = N // NCORES  # 625 rows per core
STRIPE = 125
NSTRIPE = RS // STRIPE  # 5
CB = 512
NCB = (N + CB - 1) // CB  # 10
NJC = (N + 127) // 128  # 40
JPACK = 8  # transposed 128-col chunks packed per PSUM bank
HALF = NP // 2  # 2560: gather half-width (= 5 CB blocks)

_CACHE = {}


def _c_blocks():
    return [(cb * CB, min(CB, N - cb * CB)) for cb in range(NCB)]


def _j_chunks():
    return [(j, min(128, N - j * 128)) for j in range(NJC)]


def build():
    nc = bacc.Bacc(num_swdge_queues=4)

    adjg = nc.declare_dram_parameter("adjg", [KTOT, RS, NP], BF16, isOutput=False)
    idxs = nc.declare_dram_parameter("idxs", [128, 8 * NSTRIPE], mybir.dt.int16, isOutput=False)
    sst = nc.declare_dram_parameter("sst", [STRIPE, NSTRIPE * 32], BF16, isOutput=False)
    sfa = nc.declare_dram_parameter("sfa", [128, NJC * OUT], BF16, isOutput=False)
    sfb = nc.declare_dram_parameter("sfb", [128, NJC * OUT], BF16, isOutput=False)
    idt = nc.declare_dram_parameter("idt", [128, 128], BF16, isOutput=False)

    o1 = nc.declare_dram_parameter("o1", [48, N], BF16, isOutput=True)
    o2a = nc.declare_dram_parameter("o2a", [OUT, RS], F32, isOutput=True)
    o2b = nc.declare_dram_parameter("o2b", [OUT, RS], F32, isOutput=True)

    # group-a relations (k<3) via sync-HWDGE; group-b via SWDGE gathers
    # spread over the 4 SWDGE queues

    with TileContext(nc) as tc:
        with (
            tc.tile_pool(name="persist", bufs=1) as pp,
            tc.tile_pool(name="rawa", bufs=5) as rawap,
            tc.tile_pool(name="rawg", bufs=19) as rawgp,
            tc.tile_pool(name="mrg", bufs=1) as mrgp,
            tc.tile_pool(name="mrgb", bufs=2) as mrgbp,
            tc.tile_pool(name="strip", bufs=3) as stripp,
            tc.tile_pool(name="pdir", bufs=3, space="PSUM") as pdirp,
            tc.tile_pool(name="pt", bufs=3, space="PSUM") as ptp,
            tc.tile_pool(name="pd2", bufs=2, space="PSUM") as pd2p,
        ):
            # ---------------- persistent tiles ----------------
            sst_t = pp.tile([STRIPE, NSTRIPE * 32], BF16, tag="sst")
            nc.sync.dma_start(out=sst_t, in_=sst[:, :])
            sfa_t = pp.tile([128, NJC * OUT], BF16, tag="sfa")
            nc.sync.dma_start(out=sfa_t, in_=sfa[:, :])
            sfb_t = pp.tile([128, NJC * OUT], BF16, tag="sfb")
            nc.sync.dma_start(out=sfb_t, in_=sfb[:, :])
            id_t = pp.tile([128, 128], BF16, tag="idt")
            nc.sync.dma_start(out=id_t, in_=idt[:, :])
            ix = pp.tile([128, 8 * NSTRIPE], mybir.dt.int16, tag="ix")
            nc.sync.dma_start(out=ix, in_=idxs[:, :])

            o1sb = pp.tile([48, N], BF16, tag="o1sb")
            acc2a = pp.tile([OUT, RS], F32, tag="acc2a")
            acc2b = pp.tile([OUT, RS], F32, tag="acc2b")

            for st in range(NSTRIPE):
                r0 = st * STRIPE
                # ---- loads: full-row gathers for group a, half-row gathers
                # (aligned to CB blocks 0-4 / 5-9) for group b ----
                # issue order matters: Q7 executes gens in order, so put
                # b-half0 (slots free mid-stripe) before group-a (slots
                # free at dir1-chain end) before b-half1
                raw = {}
                qn = st  # rotate queue assignment across stripes
                for h in range(2):
                    for k in range(K_A, KTOT):
                        t = rawgp.tile(
                            [128, 1, HALF], BF16, tag="trawg", name=f"t_{st}_{k}_{h}"
                        )
                        nc.gpsimd.dma_gather(
                            t,
                            adjg[k, :, h * HALF : (h + 1) * HALF],
                            ix[:, st * 8 : (st + 1) * 8],
                            128,
                            128,
                            HALF,
                            elem_step=NP,
                            queue_num=qn % 4,
                        )
                        qn += 1
                        raw[(k, h)] = t
                    if h == 0:
                        for k in range(K_A):
                            t = rawap.tile(
                                [128, 1, NP], BF16, tag="trawa", name=f"t_{st}_{k}"
                            )
                            nc.gpsimd.dma_gather(
                                t,
                                adjg[k, :, :],
                                ix[:, st * 8 : (st + 1) * 8],
                                128,
                                128,
                                NP,
                                elem_step=NP,
                                queue_num=qn % 4,
                            )
                            qn += 1
                            raw[k] = t

                def rawsl(k, c0, cw):
                    if k < K_A:
                        return raw[k][:STRIPE, 0, c0 : c0 + cw]
                    h = 0 if c0 < HALF else 1
                    assert c0 + cw <= HALF or c0 >= HALF
                    return raw[(k, h)][:STRIPE, 0, c0 - h * HALF : c0 - h * HALF + cw]

                # ---- dir1: two independent PSUM chains (group a from the
                # sync tile, group b from gathers) into disjoint partition
                # ranges of one bank; a completes early, freeing tsync ----
                sa = sst_t[:, st * 32 : st * 32 + OUT]
                sb = sst_t[:, st * 32 + OUT : st * 32 + 32]
                for cb, (c0, cw) in enumerate(_c_blocks()):
                    pd = pdirp.tile([48, CB], F32, tag="pd", name=f"pd_{st}_{cb}")
                    for k in range(K_A):
                        nc.tensor.matmul(
                            pd[0:OUT, :cw],
                            sa,
                            rawsl(k, c0, cw),
                            start=(k == 0),
                            stop=(k == K_A - 1),
                        )
                    for k in range(K_A, KTOT):
                        nc.tensor.matmul(
                            pd[32:48, :cw],
                            sb,
                            rawsl(k, c0, cw),
                            start=(k == K_A),
                            stop=(k == KTOT - 1),
                        )
                    dst = o1sb[:, c0 : c0 + cw]
                    if st == 0:
                        nc.vector.tensor_copy(out=dst, in_=pd[:, :cw])
                    else:
                        nc.vector.tensor_add(dst, dst, pd[:, :cw])

                # ---- merge per group: mrg = sum_k (w_k A_k) (bf16 adds) ----
                mrga = mrgp.tile([STRIPE, N], BF16, tag="mrga", name=f"mrga_{st}")
                mrgb = mrgbp.tile([STRIPE, N], BF16, tag="mrgb", name=f"mrgb_{st}")
                nc.vector.tensor_add(mrga, rawsl(0, 0, N), rawsl(1, 0, N))
                nc.vector.tensor_add(mrga, mrga, rawsl(2, 0, N))
                for h, c0, cw in ((0, 0, HALF), (1, HALF, N - HALF)):
                    dst = mrgb[:, c0 : c0 + cw]
                    nc.vector.tensor_add(
                        dst, rawsl(3, c0, cw), rawsl(4, c0, cw)
                    )
                    for k in range(5, KTOT):
                        nc.vector.tensor_add(dst, dst, rawsl(k, c0, cw))

                # ---- dir2 per group: acc2[:, st] = (mrg @ S)^T ----
                for gname, mrg, sf_t, acc2 in (
                    ("a", mrga, sfa_t, acc2a),
                    ("b", mrgb, sfb_t, acc2b),
                ):
                    pd2 = pd2p.tile(
                        [OUT, 126], F32, tag="pd2", name=f"pd2_{st}_{gname}"
                    )
                    jcs = _j_chunks()
                    for jb in range(0, NJC, JPACK):
                        chunk = jcs[jb : jb + JPACK]
                        pt = ptp.tile(
                            [128, JPACK * 126],
                            BF16,
                            tag="pt",
                            name=f"pt_{st}_{gname}_{jb}",
                        )
                        for jj, (j, cjw) in enumerate(chunk):
                            nc.tensor.transpose(
                                pt[:cjw, jj * 126 : jj * 126 + 126],
                                mrg[:STRIPE, 128 * j : 128 * j + cjw],
                                id_t[:STRIPE, :126],
                            )
                        strip = stripp.tile(
                            [128, JPACK * 126],
                            BF16,
                            tag="tt",
                            name=f"tt_{st}_{gname}_{jb}",
                        )
                        wid = len(chunk) * 126
                        nc.scalar.copy(out=strip[:, :wid], in_=pt[:, :wid])
                        for jj, (j, cjw) in enumerate(chunk):
                            nc.tensor.matmul(
                                pd2[:, :126],
                                sf_t[:cjw, j * OUT : (j + 1) * OUT],
                                strip[:cjw, jj * 126 : jj * 126 + 126],
                                start=(j == 0),
                                stop=(j == NJC - 1),
                            )
                    nc.vector.tensor_copy(
                        out=acc2[:, r0 : r0 + STRIPE], in_=pd2[:, :STRIPE]
                    )
                    o2 = o2a if gname == "a" else o2b
                    nc.sync.dma_start(
                        out=o2[:, r0 : r0 + STRIPE],
                        in_=acc2[:, r0 : r0 + STRIPE],
                    )

            nc.sync.dma_start(out=o1[:, :], in_=o1sb)

    nc.compile()
    return nc


def _make_inputs(feature, A, A_t, w2, wb, W3, W1):
    bf16 = ml_dtypes.bfloat16

    S3 = (feature @ W3).astype(np.float32)  # [N, 16]
    S1 = (feature @ W1).astype(np.float32)

    # sf: S laid out [128, NJC*16]; sf[p, j*16+o] = S[j*128+p, o]
    def make_sf(S):
        sf = np.zeros((128, NJC * OUT), dtype=np.float32)
        for j in range(NJC):
            w = min(128, N - j * 128)
            sf[:w, j * OUT : (j + 1) * OUT] = S[j * 128 : j * 128 + w]
        return sf.astype(bf16)

    sfa = make_sf(S3)
    sfb = make_sf(S1)
    eye = np.eye(128, dtype=bf16)

    # w_k folded into the adjacency cast
    Ascl = (A * w2[:, None, None]).astype(bf16)  # [3, N, N]
    Atscl = (A_t * wb[:, None, None]).astype(bf16)  # [9, N, N]

    idxs = np.full((128, 8 * NSTRIPE), -1, dtype=np.int16)
    for st in range(NSTRIPE):
        for j in range(STRIPE):
            for rep in range(8):
                idxs[j % 16 + 16 * rep, st * 8 + j // 16] = STRIPE * st + j

    in_maps = []
    for p in range(NCORES):
        r0 = p * RS
        adj = np.concatenate(
            [Ascl[:, r0 : r0 + RS, :], Atscl[:, r0 : r0 + RS, :]], axis=0
        )
        adjg = np.zeros((KTOT, RS, NP), dtype=bf16)
        adjg[:, :, :N] = adj
        # dir1 stationaries: [125, st*32 + (0:16 S3 | 16:32 S1)]
        sst = np.zeros((STRIPE, NSTRIPE * 32), dtype=np.float32)
        for st in range(NSTRIPE):
            rows = slice(r0 + st * STRIPE, r0 + (st + 1) * STRIPE)
            sst[:, st * 32 : st * 32 + OUT] = S3[rows]
            sst[:, st * 32 + OUT : st * 32 + 32] = S1[rows]
        in_maps.append(
            {
                "adjg": adjg,
                "idxs": idxs,
                "sst": sst.astype(bf16),
                "sfa": sfa,
                "sfb": sfb,
                "idt": eye,
            }
        )
    return in_maps


def kernel(feature, A, A_t, weight_b2, weight_b, W3, b3, W1, b1, **kw):
    feature = np.asarray(feature, dtype=np.float32)
    A = np.asarray(A, dtype=np.float32)
    A_t = np.asarray(A_t, dtype=np.float32)
    w2 = np.asarray(weight_b2, dtype=np.float32).reshape(K_A)
    wb = np.asarray(weight_b, dtype=np.float32).reshape(K_AT)
    W3 = np.asarray(W3, dtype=np.float32)
    W1 = np.asarray(W1, dtype=np.float32)
    b3 = np.asarray(b3, dtype=np.float32)
    b1 = np.asarray(b1, dtype=np.float32)

    if "nc" not in _CACHE:
        _CACHE["nc"] = build()
    nc = _CACHE["nc"]

    in_maps = _make_inputs(feature, A, A_t, w2, wb, W3, W1)
    _CACHE["in_maps"] = in_maps

    res = run_bass_kernel_spmd(nc, in_maps, core_ids=list(range(NCORES)))

    col_a = np.zeros((OUT, N), dtype=np.float32)
    col_b = np.zeros((OUT, N), dtype=np.float32)
    row_a = np.empty((OUT, N), dtype=np.float32)
    row_b = np.empty((OUT, N), dtype=np.float32)
    for p in range(NCORES):
        r = res.results[p]
        col_a += r["o1"][0:16].astype(np.float32)
        col_b += r["o1"][32:48].astype(np.float32)
        row_a[:, p * RS : (p + 1) * RS] = r["o2a"]
        row_b[:, p * RS : (p + 1) * RS] = r["o2b"]

    U1 = (col_a + row_a).T + b3
    U2 = (col_b + row_b).T + b1
    return np.concatenate([U1, U2], axis=1).astype(np.float32)


# revision 55
# speedup vs baseline: 2.4347x; 1.0077x over previous
"""FAME-GCN Trainium2 kernel, v3.

Computes, for merged adjacency final_A = temp + temp^T, temp = sum_k w_k A_k:
    U1 = final_A @ (feature @ W3) + b3
    U2 = final_A2 @ (feature @ W1) + b1
    out = concat(U1, U2, axis=1)          # [5000, 32]

Distribution: node rows sharded 625/core across 8 NeuronCores; the [16, N]
column-direction partials (temp^T S) are summed across cores on the host,
the row-direction results (temp S)[own rows] concatenate.

Host prep: w_k is folded into a bf16 cast of each adjacency (w_k * A_k),
so the on-device merge is a pure chain of 2x-mode bf16 tensor_adds and the
dir1 stationaries are plain support matrices.

Per core, 5 stripes of 125 rows:
  - the 12 scaled relation stripes arrive as [125, 5000] bf16 transfers
    spread over six DMA paths (sync-HWDGE, scalar-HWDGE, 4 SWDGE queues),
  - dir1 (temp^T S, both groups): per-relation matmuls against a stacked
    [125, 32] stationary (S3-half for A-relations, S1-half for A_t),
    PSUM-accumulated over all 12 relations, one flush per column block,
  - dir2 (temp S): bf16 tensor_add merge per group on DVE, then PE
    transposes (8 column chunks packed per PSUM bank, bulk-copied to SBUF
    by the scalar engine) feeding an S-stationary accumulation chain.
"""

import sys

if "/opt/trn_rl_repo" not in sys.path:
    sys.path.insert(0, "/opt/trn_rl_repo")

import ml_dtypes
import numpy as np

import concourse.bacc as bacc
import concourse.mybir as mybir
from concourse.tile import TileContext
from concourse.bass_utils import run_bass_kernel_spmd

F32 = mybir.dt.float32
BF16 = mybir.dt.bfloat16

N = 5000
NP = 5120  # padded row length for the gather path (row bytes % 256 == 0)
OUT = 16
K_A, K_AT = 3, 9
KTOT = K_A + K_AT
NCORES = 8
RS = N // NCORES  # 625 rows per core
STRIPE = 125
NSTRIPE = RS // STRIPE  # 5
CB = 512
NCB = (N + CB - 1) // CB  # 10
NJC = (N + 127) // 128  # 40
JPACK = 8  # transposed 128-col chunks packed per PSUM bank
HALF = NP // 2  # 2560: gather half-width (= 5 CB blocks)

_CACHE = {}


def _c_blocks():
    return [(cb * CB, min(CB, N - cb * CB)) for cb in range(NCB)]


def _j_chunks():
    return [(j, min(128, N - j * 128)) for j in range(NJC)]


def build():
    nc = bacc.Bacc(num_swdge_queues=4)

    adjg = nc.declare_dram_parameter("adjg", [KTOT, RS, NP], BF16, isOutput=False)
    idxs = nc.declare_dram_parameter("idxs", [128, 8 * NSTRIPE], mybir.dt.int16, isOutput=False)
    sst = nc.declare_dram_parameter("sst", [STRIPE, NSTRIPE * 32], BF16, isOutput=False)
    sfa = nc.declare_dram_parameter("sfa", [128, NJC * OUT], BF16, isOutput=False)
    sfb = nc.declare_dram_parameter("sfb", [128, NJC * OUT], BF16, isOutput=False)
    idt = nc.declare_dram_parameter("idt", [128, 128], BF16, isOutput=False)

    o1 = nc.declare_dram_parameter("o1", [48, N], BF16, isOutput=True)
    o2a = nc.declare_dram_parameter("o2a", [OUT, RS], F32, isOutput=True)
    o2b = nc.declare_dram_parameter("o2b", [OUT, RS], F32, isOutput=True)

    # group-a relations (k<3) via sync-HWDGE; group-b via SWDGE gathers
    # spread over the 4 SWDGE queues

    with TileContext(nc) as tc:
        with (
            tc.tile_pool(name="persist", bufs=1) as pp,
            tc.tile_pool(name="rawa", bufs=5) as rawap,
            tc.tile_pool(name="rawg", bufs=20) as rawgp,
            tc.tile_pool(name="mrg", bufs=1) as mrgp,
            tc.tile_pool(name="mrgb", bufs=2) as mrgbp,
            tc.tile_pool(name="strip", bufs=3) as stripp,
            tc.tile_pool(name="pdir", bufs=3, space="PSUM") as pdirp,
            tc.tile_pool(name="pt", bufs=3, space="PSUM") as ptp,
            tc.tile_pool(name="pd2", bufs=2, space="PSUM") as pd2p,
        ):
            # ---------------- persistent tiles ----------------
            sst_t = pp.tile([STRIPE, NSTRIPE * 32], BF16, tag="sst")
            nc.sync.dma_start(out=sst_t, in_=sst[:, :])
            sfa_t = pp.tile([128, NJC * OUT], BF16, tag="sfa")
            nc.sync.dma_start(out=sfa_t, in_=sfa[:, :])
            sfb_t = pp.tile([128, NJC * OUT], BF16, tag="sfb")
            nc.sync.dma_start(out=sfb_t, in_=sfb[:, :])
            id_t = pp.tile([128, 128], BF16, tag="idt")
            nc.sync.dma_start(out=id_t, in_=idt[:, :])
            ix = pp.tile([128, 8 * NSTRIPE], mybir.dt.int16, tag="ix")
            nc.sync.dma_start(out=ix, in_=idxs[:, :])

            o1sb = pp.tile([48, N], BF16, tag="o1sb")
            acc2a = pp.tile([OUT, RS], F32, tag="acc2a")
            acc2b = pp.tile([OUT, RS], F32, tag="acc2b")

            for st in range(NSTRIPE):
                r0 = st * STRIPE
                # ---- loads: full-row gathers for group a, half-row gathers
                # (aligned to CB blocks 0-4 / 5-9) for group b ----
                # issue order matters: Q7 executes gens in order, so put
                # b-half0 (slots free mid-stripe) before group-a (slots
                # free at dir1-chain end) before b-half1
                raw = {}
                qn = st  # rotate queue assignment across stripes
                for h in range(2):
                    for k in range(K_A, KTOT):
                        t = rawgp.tile(
                            [128, 1, HALF], BF16, tag="trawg", name=f"t_{st}_{k}_{h}"
                        )
                        nc.gpsimd.dma_gather(
                            t,
                            adjg[k, :, h * HALF : (h + 1) * HALF],
                            ix[:, st * 8 : (st + 1) * 8],
                            128,
                            128,
                            HALF,
                            elem_step=NP,
                            queue_num=qn % 4,
                        )
                        qn += 1
                        raw[(k, h)] = t
                    if h == 0:
                        for k in range(K_A):
                            t = rawap.tile(
                                [128, 1, NP], BF16, tag="trawa", name=f"t_{st}_{k}"
                            )
                            nc.gpsimd.dma_gather(
                                t,
                                adjg[k, :, :],
                                ix[:, st * 8 : (st + 1) * 8],
                                128,
                                128,
                                NP,
                                elem_step=NP,
                                queue_num=qn % 4,
                            )
                            qn += 1
                            raw[k] = t

                def rawsl(k, c0, cw):
                    if k < K_A:
                        return raw[k][:STRIPE, 0, c0 : c0 + cw]
                    h = 0 if c0 < HALF else 1
                    assert c0 + cw <= HALF or c0 >= HALF
                    return raw[(k, h)][:STRIPE, 0, c0 - h * HALF : c0 - h * HALF + cw]

                # ---- dir1: two independent PSUM chains (group a from the
                # sync tile, group b from gathers) into disjoint partition
                # ranges of one bank; a completes early, freeing tsync ----
                sa = sst_t[:, st * 32 : st * 32 + OUT]
                sb = sst_t[:, st * 32 + OUT : st * 32 + 32]
                for cb, (c0, cw) in enumerate(_c_blocks()):
                    pd = pdirp.tile([48, CB], F32, tag="pd", name=f"pd_{st}_{cb}")
                    for k in range(K_A):
                        nc.tensor.matmul(
                            pd[0:OUT, :cw],
                            sa,
                            rawsl(k, c0, cw),
                            start=(k == 0),
                            stop=(k == K_A - 1),
                        )
                    for k in range(K_A, KTOT):
                        nc.tensor.matmul(
                            pd[32:48, :cw],
                            sb,
                            rawsl(k, c0, cw),
                            start=(k == K_A),
                            stop=(k == KTOT - 1),
                        )
                    dst = o1sb[:, c0 : c0 + cw]
                    if st == 0:
                        nc.vector.tensor_copy(out=dst, in_=pd[:, :cw])
                    else:
                        nc.vector.tensor_add(dst, dst, pd[:, :cw])

                # ---- merge per group: mrg = sum_k (w_k A_k) (bf16 adds) ----
                mrga = mrgp.tile([STRIPE, N], BF16, tag="mrga", name=f"mrga_{st}")
                mrgb = mrgbp.tile([STRIPE, N], BF16, tag="mrgb", name=f"mrgb_{st}")
                nc.vector.tensor_add(mrga, rawsl(0, 0, N), rawsl(1, 0, N))
                nc.vector.tensor_add(mrga, mrga, rawsl(2, 0, N))
                for h, c0, cw in ((0, 0, HALF), (1, HALF, N - HALF)):
                    dst = mrgb[:, c0 : c0 + cw]
                    nc.vector.tensor_add(
                        dst, rawsl(3, c0, cw), rawsl(4, c0, cw)
                    )
                    for k in range(5, KTOT):
                        nc.vector.tensor_add(dst, dst, rawsl(k, c0, cw))

                # ---- dir2 per group: acc2[:, st] = (mrg @ S)^T ----
                for gname, mrg, sf_t, acc2 in (
                    ("a", mrga, sfa_t, acc2a),
                    ("b", mrgb, sfb_t, acc2b),
                ):
                    pd2 = pd2p.tile(
                        [OUT, 126], F32, tag="pd2", name=f"pd2_{st}_{gname}"
                    )
                    jcs = _j_chunks()
                    for jb in range(0, NJC, JPACK):
                        chunk = jcs[jb : jb + JPACK]
                        pt = ptp.tile(
                            [128, JPACK * 126],
                            BF16,
                            tag="pt",
                            name=f"pt_{st}_{gname}_{jb}",
                        )
                        for jj, (j, cjw) in enumerate(chunk):
                            nc.tensor.transpose(
                                pt[:cjw, jj * 126 : jj * 126 + 126],
                                mrg[:STRIPE, 128 * j : 128 * j + cjw],
                                id_t[:STRIPE, :126],
                            )
                        strip = stripp.tile(
                            [128, JPACK * 126],
                            BF16,
                            tag="tt",
                            name=f"tt_{st}_{gname}_{jb}",
                        )
                        wid = len(chunk) * 126
                        nc.scalar.copy(out=strip[:, :wid], in_=pt[:, :wid])
                        for jj, (j, cjw) in enumerate(chunk):
                            nc.tensor.matmul(
                                pd2[:, :126],
                                sf_t[:cjw, j * OUT : (j + 1) * OUT],
                                strip[:cjw, jj * 126 : jj * 126 + 126],
                                start=(j == 0),
                                stop=(j == NJC - 1),
                            )
                    nc.vector.tensor_copy(
                        out=acc2[:, r0 : r0 + STRIPE], in_=pd2[:, :STRIPE]
                    )
                    o2 = o2a if gname == "a" else o2b
                    nc.sync.dma_start(
                        out=o2[:, r0 : r0 + STRIPE],
                        in_=acc2[:, r0 : r0 + STRIPE],
                    )

            nc.sync.dma_start(out=o1[:, :], in_=o1sb)

    nc.compile()
    return nc


def _make_inputs(feature, A, A_t, w2, wb, W3, W1):
    bf16 = ml_dtypes.bfloat16

    S3 = (feature @ W3).astype(np.float32)  # [N, 16]
    S1 = (feature @ W1).astype(np.float32)

    # sf: S laid out [128, NJC*16]; sf[p, j*16+o] = S[j*128+p, o]
    def make_sf(S):
        sf = np.zeros((128, NJC * OUT), dtype=np.float32)
        for j in range(NJC):
            w = min(128, N - j * 128)
            sf[:w, j * OUT : (j + 1) * OUT] = S[j * 128 : j * 128 + w]
        return sf.astype(bf16)

    sfa = make_sf(S3)
    sfb = make_sf(S1)
    eye = np.eye(128, dtype=bf16)

    # w_k folded into the adjacency cast
    Ascl = (A * w2[:, None, None]).astype(bf16)  # [3, N, N]
    Atscl = (A_t * wb[:, None, None]).astype(bf16)  # [9, N, N]

    idxs = np.full((128, 8 * NSTRIPE), -1, dtype=np.int16)
    for st in range(NSTRIPE):
        for j in range(STRIPE):
            for rep in range(8):
                idxs[j % 16 + 16 * rep, st * 8 + j // 16] = STRIPE * st + j

    in_maps = []
    for p in range(NCORES):
        r0 = p * RS
        adj = np.concatenate(
            [Ascl[:, r0 : r0 + RS, :], Atscl[:, r0 : r0 + RS, :]], axis=0
        )
        adjg = np.zeros((KTOT, RS, NP), dtype=bf16)
        adjg[:, :, :N] = adj
        # dir1 stationaries: [125, st*32 + (0:16 S3 | 16:32 S1)]
        sst = np.zeros((STRIPE, NSTRIPE * 32), dtype=np.float32)
        for st in range(NSTRIPE):
            rows = slice(r0 + st * STRIPE, r0 + (st + 1) * STRIPE)
            sst[:, st * 32 : st * 32 + OUT] = S3[rows]
            sst[:, st * 32 + OUT : st * 32 + 32] = S1[rows]
        in_maps.append(
            {
                "adjg": adjg,
                "idxs": idxs,
                "sst": sst.astype(bf16),
                "sfa": sfa,
                "sfb": sfb,
                "idt": eye,
            }
        )
    return in_maps


def kernel(feature, A, A_t, weight_b2, weight_b, W3, b3, W1, b1, **kw):
    feature = np.asarray(feature, dtype=np.float32)
    A = np.asarray(A, dtype=np.float32)
    A_t = np.asarray(A_t, dtype=np.float32)
    w2 = np.asarray(weight_b2, dtype=np.float32).reshape(K_A)
    wb = np.asarray(weight_b, dtype=np.float32).reshape(K_AT)
    W3 = np.asarray(W3, dtype=np.float32)
    W1 = np.asarray(W1, dtype=np.float32)
    b3 = np.asarray(b3, dtype=np.float32)
    b1 = np.asarray(b1, dtype=np.float32)

    if "nc" not in _CACHE:
        _CACHE["nc"] = build()
    nc = _CACHE["nc"]

    in_maps = _make_inputs(feature, A, A_t, w2, wb, W3, W1)
    _CACHE["in_maps"] = in_maps

    res = run_bass_kernel_spmd(nc, in_maps, core_ids=list(range(NCORES)))

    col_a = np.zeros((OUT, N), dtype=np.float32)
    col_b = np.zeros((OUT, N), dtype=np.float32)
    row_a = np.empty((OUT, N), dtype=np.float32)
    row_b = np.empty((OUT, N), dtype=np.float32)
    for p in range(NCORES):
        r = res.results[p]
        col_a += r["o1"][0:16].astype(np.float32)
        col_b += r["o1"][32:48].astype(np.float32)
        row_a[:, p * RS : (p + 1) * RS] = r["o2a"]
        row_b[:, p * RS : (p + 1) * RS] = r["o2b"]

    U1 = (col_a + row_a).T + b3
    U2 = (col_b + row_b).T + b1
    return np.concatenate([U1, U2], axis=1).astype(np.float32)
